# revision 39
# baseline (speedup 1.0000x reference)
"""Trainium2 Bass kernel for nn_MoEPairwise (MoE pairwise routing rollout).

Strategy
--------
Data-parallel over batch B=8 == 8 NeuronCores; zero collectives. Each core
runs the full 2-step rollout for one batch element (K=32 agents, D=256,
E=8 experts).

Math factorization (validated vs reference in numpy + CoreSim):
  - pair_in = [si, sj] so expert GEMM1 factors: h1_e = relu(A_e[.,i] + B_e[.,j] + b1)
    with A_e = W1si_e^T h, B_e = W1sj_e^T h  (GEMMs over 32 tokens, not 1024 pairs).
    A^T/B^T are computed token-major (lhsT = h) and the (i,j) broadcast-add runs
    on the PE as [A^T;B^T]^T @ SEL2 with a constant 0/1 selector, writing the
    1024-wide pre-activation straight into PSUM. b1 folds into the relu-evict
    bias (per-partition), w-multiply is a dense bf16 DVE op against a
    DMA-broadcast weight plane.
  - softmax weights w >= 0, so the weighted reduction folds over the pair grid:
    delta_inter = sum_e W2_e^T (sum_i w_e(j,i) * h1_e[:, (j,i)]) + b2^T s.
    i is folded 32->8 by two dense DVE adds, then contracted on the PE with
    8 accumulating matmuls per (e,kc,mc) into per-mc PSUM tiles (PSUM
    accumulation groups must not share banks).
  - router GEMM1 factors the si/sj/si-sj rows the same way; only the si*sj
    term needs a real GEMM over the 1024-pair grid.

Layouts: feature-major activations [128 partitions = d % 128, chunk, token].
Pair-grid columns ordered (j outer, i inner). bf16 for the wide expert/router
path (PE fp32 is 4x slower; DVE tensor ops 2x faster in bf16), fp32 for the
narrow backbone (LN, small MLPs) and all PSUM accumulation.
"""

import numpy as np
import ml_dtypes

import concourse.bass as bass
import concourse.mybir as mybir
import concourse.tile as tile
from concourse import bacc
from concourse.masks import make_identity
from concourse.tile_rust import add_dep_helper

F32 = mybir.dt.float32
BF16 = mybir.dt.bfloat16
AX = mybir.AxisListType
ALU = mybir.AluOpType
ACTF = mybir.ActivationFunctionType

K = 32          # agents (tokens per batch)
D = 256         # hidden
E = 8           # experts
DS = 8          # state dim
P = 128         # partitions
C = D // P      # feature chunks (2)
EPS = 1e-5
NFOLD = 16      # folded i-groups entering the G2 contraction

BF = np.dtype(ml_dtypes.bfloat16)

# packed-constant layouts: name -> (view_shape, ncols) appended in order.
# pack tiles are [npart, total_cols]; views slice columns and rearrange.
PACK_F32A = [         # [128, .] fp32, needed early (enc/router/fs)
    ('enc_w2', (P, C, D)), ('rt_wa', (P, C, D)), ('rt_wb', (P, C, D)),
    ('fs_w1', (P, C, D)), ('fs_w2', (P, C, D)),
    ('fs_g', (P, C)), ('fs_bb', (P, C)), ('sel', (P, 4)),
]
PACK_F32B = [         # [128, .] fp32, needed late (experts/update/dec)
    ('up_w1', (P, 2 * C, D)), ('up_w2', (P, C, D)),
    ('dec_w1', (P, C, D)), ('dec_w2', (P, C, DS)),
    ('up_g', (P, 2 * C)), ('up_bb', (P, 2 * C)),
    ('ex_b1', (P, C, E)),
]
PACK_RT16 = [         # [128, .] bf16, router
    ('rt_wd', (P, C, D)), ('rt_w2', (P, C, E)),
]
PACK_EX16 = [         # [128, .] bf16, experts
    ('ex_wsij', (P, C, E, 2 * D)), ('ex_w2', (P, C, E, D)),
]
PACK_P8 = [           # [8, .] fp32
    ('enc_w1', (DS, D)), ('ex_b2t', (E, D)),
]
PACK_ROWB = [         # [1, .] bf16
    ('ex_b1z', (1, E, 2 * D)),
]
PACK_ROW = [          # [1, .] fp32
    ('enc_b1r', (1, D)), ('enc_b2r', (1, D)),
    ('fs_b1r', (1, D)), ('fs_b2r', (1, D)),
    ('rt_b1r', (1, D)), ('rt_b2r', (1, E)),
    ('up_b1r', (1, D)), ('up_b2r', (1, D)),
    ('dec_b1r', (1, D)), ('dec_b2r', (1, DS)),
]


def _cols(shape):
    n = 1
    for s in shape[1:]:
        n *= s
    return n


# --------------------------------------------------------------------------
# host-side parameter preprocessing
# --------------------------------------------------------------------------

def prep_params(params):
    p = {k: np.asarray(v, np.float32) for k, v in params.items()}
    t = {}

    def chunked(x, cin=None):  # [d_in, d_out] -> [128, cin, d_out]
        d_in = x.shape[0]
        cin = d_in // P
        return x.reshape(cin, P, -1).transpose(1, 0, 2)

    t['enc_w1'] = p['enc_W1']
    t['enc_w2'] = chunked(p['enc_W2'])
    t['fs_w1'] = chunked(p['fs_W1'])
    t['fs_w2'] = chunked(p['fs_W2'])
    Wa = p['rt_W1'][0:D] + p['rt_W1'][2 * D:3 * D]
    Wb = p['rt_W1'][D:2 * D] - p['rt_W1'][2 * D:3 * D]
    t['rt_wa'] = chunked(Wa)
    t['rt_wb'] = chunked(Wb)
    t['rt_wd'] = chunked(p['rt_W1'][3 * D:4 * D])
    t['rt_w2'] = chunked(p['rt_W2'])
    wsi = p['ex_W1'][:, 0:D, :].reshape(E, C, P, D).transpose(2, 1, 0, 3)
    wsj = p['ex_W1'][:, D:2 * D, :].reshape(E, C, P, D).transpose(2, 1, 0, 3)
    t['ex_wsij'] = np.concatenate([wsi, wsj], axis=-1)   # [128, C, E, 512]
    t['ex_w2'] = p['ex_W2'].reshape(E, C, P, D).transpose(2, 1, 0, 3)
    t['ex_b1'] = p['ex_b1'].T.reshape(C, P, E).transpose(1, 0, 2)   # [128, C, E]
    t['ex_b1z'] = np.concatenate([np.zeros_like(p['ex_b1']), p['ex_b1']],
                                 axis=1)[None]                      # [1, E, 512]
    t['ex_b2t'] = p['ex_b2']
    t['up_w1'] = chunked(p['up_W1'])
    t['up_w2'] = chunked(p['up_W2'])
    t['dec_w1'] = chunked(p['dec_W1'])
    t['dec_w2'] = chunked(p['dec_W2'])
    t['fs_g'] = p['fs_g'].reshape(C, P).T
    t['fs_bb'] = p['fs_b'].reshape(C, P).T
    t['up_g'] = p['up_g'].reshape(2 * C, P).T
    t['up_bb'] = p['up_b'].reshape(2 * C, P).T
    for n_, b_ in [('enc_b1r', 'enc_b1'), ('enc_b2r', 'enc_b2'),
                   ('fs_b1r', 'fs_b1'), ('fs_b2r', 'fs_b2'),
                   ('rt_b1r', 'rt_b1'), ('rt_b2r', 'rt_b2'),
                   ('up_b1r', 'up_b1'), ('up_b2r', 'up_b2'),
                   ('dec_b1r', 'dec_b1'), ('dec_b2r', 'dec_b2')]:
        t[n_] = p[b_][None, :]
    sel = np.zeros((P, 4), np.float32)
    for jl in range(4):
        sel[jl * K:(jl + 1) * K, jl] = 1.0
    t['sel'] = sel

    def pack(layout, npart, dtype):
        cols = sum(_cols(sh) for _, sh in layout)
        buf = np.zeros((npart, cols), np.float32)
        off = 0
        for name, sh in layout:
            n = _cols(sh)
            arr = t[name]
            assert tuple(arr.shape) == tuple(sh), (name, arr.shape, sh)
            buf[:sh[0], off:off + n] = arr.reshape(sh[0], n)
            off += n
        return np.ascontiguousarray(buf.astype(dtype))

    ins = {
        'pack_f32a': pack(PACK_F32A, P, np.float32),
        'pack_f32b': pack(PACK_F32B, P, np.float32),
        'pack_rt16': pack(PACK_RT16, P, BF),
        'pack_ex16': pack(PACK_EX16, P, BF),
        'pack_p8': pack(PACK_P8, DS, np.float32),
        'pack_row': pack(PACK_ROW, 1, np.float32),
        'pack_rowb': pack(PACK_ROWB, 1, BF),
    }
    # SEL2 [64, 1024]: row k<32 selects i==k; row k>=32 selects j==k-32
    sel2 = np.zeros((2 * K, K * K), np.float32)
    for i in range(K):
        sel2[i, i::K] = 1.0          # columns (j, i) with that i
        sel2[K + i, i * K:(i + 1) * K] = 1.0   # columns with j == i
    ins['sel2'] = np.ascontiguousarray(sel2.astype(BF))
    return ins


# --------------------------------------------------------------------------
# device program
# --------------------------------------------------------------------------

def build_program(n_steps=2, dbg=False):
    nc = bacc.Bacc("TRN2", target_bir_lowering=False, debug=False)

    dr = {}
    specs = {
        'x0t': ([DS, K], F32),
        'pack_f32a': ([P, sum(_cols(s) for _, s in PACK_F32A)], F32),
        'pack_f32b': ([P, sum(_cols(s) for _, s in PACK_F32B)], F32),
        'pack_rt16': ([P, sum(_cols(s) for _, s in PACK_RT16)], BF16),
        'pack_ex16': ([P, sum(_cols(s) for _, s in PACK_EX16)], BF16),
        'pack_p8': ([DS, sum(_cols(s) for _, s in PACK_P8)], F32),
        'pack_row': ([1, sum(_cols(s) for _, s in PACK_ROW)], F32),
        'pack_rowb': ([1, sum(_cols(s) for _, s in PACK_ROWB)], BF16),
        'sel2': ([2 * K, K * K], BF16),
    }
    for name, (shape, dt) in specs.items():
        dr[name] = nc.dram_tensor(name, shape, dt, kind="ExternalInput").ap()
    pred_out = nc.dram_tensor('pred', [n_steps, K, DS], F32, kind="ExternalOutput").ap()
    w_dram = nc.dram_tensor('w_scratch', [n_steps, E, E * P], BF16, kind="Internal").ap()
    dbg_out = None
    if dbg:
        dbg_out = {
            'dbg_h0': nc.dram_tensor('dbg_h0', [P, C, K], F32, kind="ExternalOutput").ap(),
            'dbg_w': nc.dram_tensor('dbg_w', [E, E * P], F32, kind="ExternalOutput").ap(),
            'dbg_xu': nc.dram_tensor('dbg_xu', [P, 2 * C, K], F32, kind="ExternalOutput").ap(),
            'dbg_r1': nc.dram_tensor('dbg_r1', [P, C, K, K], F32, kind="ExternalOutput").ap(),
            'dbg_prod': nc.dram_tensor('dbg_prod', [P, C, K, K], F32, kind="ExternalOutput").ap(),
            'dbg_d0': nc.dram_tensor('dbg_d0', [P, C, K], F32, kind="ExternalOutput").ap(),
        }

    with tile.TileContext(nc) as tc:
        _emit(nc, tc, dr, pred_out, n_steps, dbg_out, w_dram)
    nc.compile()
    return nc


def _emit(nc, tc, dr, pred_out, n_steps, dbg_out=None, w_dram=None):
    from contextlib import ExitStack
    ctx = ExitStack()
    with ctx:
        const = ctx.enter_context(tc.tile_pool(name="const", bufs=1))
        act = ctx.enter_context(tc.tile_pool(name="act", bufs=2))
        big = ctx.enter_context(tc.tile_pool(name="big", bufs=4))
        wpool = ctx.enter_context(tc.tile_pool(name="wpool", bufs=2))
        sm = ctx.enter_context(tc.tile_pool(name="sm", bufs=4))
        dbgp = ctx.enter_context(tc.tile_pool(name="dbgp", bufs=1))
        ps = ctx.enter_context(tc.tile_pool(name="ps", bufs=3, space="PSUM"))
        psb = ctx.enter_context(tc.tile_pool(name="psb", bufs=2, space="PSUM"))
        psd = ctx.enter_context(tc.tile_pool(name="psd", bufs=1, space="PSUM"))

        # ---- load packed constants, build per-name views
        packs = {}
        for pname in ('x0t', 'pack_p8', 'pack_f32a', 'pack_rt16', 'pack_row',
                      'sel2', 'pack_ex16', 'pack_f32b', 'pack_rowb'):
            ap = dr[pname]
            t_ = const.tile(ap.shape, ap.dtype, tag=pname)
            nc.sync.dma_start(out=t_[:], in_=ap[:])
            packs[pname] = t_

        w = {}
        for pname, layout in [('pack_f32a', PACK_F32A), ('pack_f32b', PACK_F32B),
                              ('pack_rt16', PACK_RT16), ('pack_ex16', PACK_EX16),
                              ('pack_p8', PACK_P8), ('pack_row', PACK_ROW),
                              ('pack_rowb', PACK_ROWB)]:
            off = 0
            for name, sh in layout:
                n = _cols(sh)
                view = packs[pname][:sh[0], off:off + n]
                if len(sh) == 3:
                    view = view.rearrange("p (a b) -> p a b", a=sh[1])
                elif len(sh) == 4:
                    view = view.rearrange("p (a b c) -> p a b c", a=sh[1], b=sh[2])
                w[name] = view
                off += n
        sel2 = packs['sel2']

        ones_col = const.tile([P, 1], F32)      # partition-sum matmuls
        nc.vector.memset(ones_col[:], 1.0)
        ones_row = const.tile([1, P], F32)      # partition-broadcast matmuls
        nc.vector.memset(ones_row[:], 1.0)
        ones_tok = const.tile([1, K], F32)      # bias-fold matmuls
        nc.vector.memset(ones_tok[:], 1.0)
        ones_tok_bf = const.tile([1, K], BF16)
        nc.vector.memset(ones_tok_bf[:], 1.0)
        ident = const.tile([P, P], F32)
        make_identity(nc, ident)
        eps_t = const.tile([1, 1], F32)
        nc.vector.memset(eps_t[:], EPS)

        h_f = act.tile([P, C, K], F32, tag="h_f")
        h_b = act.tile([P, C, K], BF16, tag="h_b")

        # ---- helpers ------------------------------------------------------
        def bias_fold(out_ps_mc, brow, mc, width=P, stop=False):
            nc.tensor.matmul(out_ps_mc, brow[:, mc * P:mc * P + width],
                             ones_tok[:], start=False, stop=stop)

        def mlp(x, win1, b1r, win2, b2r, out_sb, cin=C):
            hp = ps.tile([P, C, K], F32, tag="gen")
            for mc in range(C):
                for kc in range(cin):
                    nc.tensor.matmul(hp[:, mc, :],
                                     win1[:, kc, mc * P:(mc + 1) * P], x[:, kc, :],
                                     start=(kc == 0), stop=False)
                bias_fold(hp[:, mc, :], b1r, mc, stop=True)
            hs = sm.tile([P, C, K], F32, tag="mlp_h")
            nc.scalar.activation(hs[:], hp[:], ACTF.Relu)
            op = ps.tile([P, C, K], F32, tag="gen")
            for mc in range(C):
                for kc in range(C):
                    nc.tensor.matmul(op[:, mc, :],
                                     win2[:, kc, mc * P:(mc + 1) * P], hs[:, kc, :],
                                     start=(kc == 0), stop=False)
                bias_fold(op[:, mc, :], b2r, mc, stop=True)
            nc.scalar.activation(out_sb, op[:], ACTF.Copy)
            return out_sb

        def layer_norm(x, g, bb, cin, out):
            sq = sm.tile([P, cin, K], F32, tag="ln_sq")
            nc.vector.tensor_mul(sq[:], x[:], x[:])
            mp = ps.tile([1, K], F32, tag="gen")
            qp = ps.tile([1, K], F32, tag="gen")
            for kc in range(cin):
                nc.tensor.matmul(mp[:], ones_col[:], x[:, kc, :],
                                 start=(kc == 0), stop=(kc == cin - 1))
            for kc in range(cin):
                nc.tensor.matmul(qp[:], ones_col[:], sq[:, kc, :],
                                 start=(kc == 0), stop=(kc == cin - 1))
            st = sm.tile([1, 2, K], F32, tag="ln_st")
            dinv = 1.0 / (cin * P)
            nc.vector.tensor_scalar(out=st[:, 0, :], in0=mp[:], scalar1=dinv,
                                    scalar2=None, op0=ALU.mult)
            m2 = sm.tile([1, K], F32, tag="ln_m2")
            nc.vector.tensor_scalar(out=m2[:], in0=qp[:], scalar1=dinv,
                                    scalar2=None, op0=ALU.mult)
            var = sm.tile([1, K], F32, tag="ln_var")
            nc.vector.tensor_mul(var[:], st[:, 0, :], st[:, 0, :])
            nc.vector.tensor_sub(var[:], m2[:], var[:])
            std = sm.tile([1, K], F32, tag="ln_std")
            nc.scalar.activation(std[:], var[:], ACTF.Sqrt, bias=eps_t[:])
            nc.vector.reciprocal(st[:, 1, :], std[:])
            bp = ps.tile([P, 2, K], F32, tag="gen")
            nc.tensor.matmul(bp[:, 0, :], ones_row[:], st[:, 0, :], start=True, stop=True)
            nc.tensor.matmul(bp[:, 1, :], ones_row[:], st[:, 1, :], start=True, stop=True)
            t1 = sm.tile([P, cin, K], F32, tag="ln_t1")
            mb = bp[:, 0:1, :].broadcast_to([P, cin, K])
            rb = bp[:, 1:2, :].broadcast_to([P, cin, K])
            nc.vector.tensor_sub(t1[:], x[:], mb)
            nc.vector.tensor_mul(t1[:], t1[:], rb)
            for c_ in range(cin):
                nc.vector.tensor_scalar(out=out[:, c_, :], in0=t1[:, c_, :],
                                        scalar1=g[:, c_:c_ + 1],
                                        scalar2=bb[:, c_:c_ + 1],
                                        op0=ALU.mult, op1=ALU.add)
            return out

        def encode(x_sb):
            hp = ps.tile([P, C, K], F32, tag="gen")
            for mc in range(C):
                nc.tensor.matmul(hp[:, mc, :], w['enc_w1'][:, mc * P:(mc + 1) * P],
                                 x_sb[:], start=True, stop=False)
                bias_fold(hp[:, mc, :], w['enc_b1r'], mc, stop=True)
            hs = sm.tile([P, C, K], F32, tag="enc_h")
            nc.scalar.activation(hs[:], hp[:], ACTF.Relu)
            op = ps.tile([P, C, K], F32, tag="gen")
            for mc in range(C):
                for kc in range(C):
                    nc.tensor.matmul(op[:, mc, :],
                                     w['enc_w2'][:, kc, mc * P:(mc + 1) * P],
                                     hs[:, kc, :], start=(kc == 0), stop=False)
                bias_fold(op[:, mc, :], w['enc_b2r'], mc, stop=True)
            nc.scalar.activation(h_f[:], op[:], ACTF.Copy)
            nc.scalar.activation(h_b[:], op[:], ACTF.Copy)

        # ---- initial encode
        x0 = packs['x0t']
        encode(x0)
        if dbg_out is not None:
            nc.sync.dma_start(out=dbg_out['dbg_h0'][:], in_=h_f[:])

        # ---- rollout steps
        for t in range(n_steps):
            # ---------- router ----------
            mulij = big.tile([P, C, K, K], BF16, tag="mulij")
            for kc in range(C):
                nc.vector.tensor_mul(
                    mulij[:, kc],
                    h_b[:, kc, None, :].broadcast_to([P, K, K]),
                    h_b[:, kc, :, None].broadcast_to([P, K, K]))

            arp = ps.tile([P, C, K], F32, tag="gen")
            brp = ps.tile([P, C, K], F32, tag="gen")
            for mc in range(C):
                for kc in range(C):
                    nc.tensor.matmul(arp[:, mc, :],
                                     w['rt_wa'][:, kc, mc * P:(mc + 1) * P],
                                     h_f[:, kc, :], start=(kc == 0), stop=False)
                bias_fold(arp[:, mc, :], w['rt_b1r'], mc, stop=True)
                for kc in range(C):
                    nc.tensor.matmul(brp[:, mc, :],
                                     w['rt_wb'][:, kc, mc * P:(mc + 1) * P],
                                     h_f[:, kc, :], start=(kc == 0),
                                     stop=(kc == C - 1))
            ar = sm.tile([P, C, K], BF16, tag="ar")
            br = sm.tile([P, C, K], BF16, tag="br")
            nc.scalar.activation(ar[:], arp[:], ACTF.Copy)
            nc.scalar.activation(br[:], brp[:], ACTF.Copy)

            r1 = big.tile([P, C, K, K], BF16, tag="r1")
            for mc in range(C):
                cp = psb.tile([P, K, K], F32, tag="pbig")
                for kc in range(C):
                    for nh in range(2):
                        nc.tensor.matmul(
                            cp[:, nh * 16:(nh + 1) * 16, :],
                            w['rt_wd'][:, kc, mc * P:(mc + 1) * P],
                            mulij[:, kc, nh * 16:(nh + 1) * 16, :],
                            start=(kc == 0), stop=(kc == C - 1))
                pre = big.tile([P, K, K], BF16, tag="rpre")
                nc.gpsimd.tensor_tensor(
                    pre[:], ar[:, mc, None, :].broadcast_to([P, K, K]),
                    br[:, mc, :, None].broadcast_to([P, K, K]), op=ALU.add)
                nc.vector.tensor_add(pre[:], pre[:], cp[:])
                nc.scalar.activation(r1[:, mc, :, :], pre[:], ACTF.Relu)

            # logits_T + softmax over free dim e, per 128-pair chunk
            wt = sm.tile([P, E, E], F32, tag="wt")
            lp = ps.tile([P, E, E], F32, tag="gen")
            nc.vector.memset(lp[:], 0.0)
            for pc in range(E):
                for kc in range(C):
                    nc.tensor.matmul(lp[:, pc, :], r1[:, kc, :, :].rearrange(
                        "p a b -> p (a b)")[:, pc * P:(pc + 1) * P],
                        w['rt_w2'][:, kc, :], start=False,
                        stop=(pc == E - 1 and kc == C - 1),
                        skip_group_check=True)
                nc.tensor.matmul(lp[:, pc, :], ones_row[:], w['rt_b2r'][:],
                                 start=False, stop=False,
                                 skip_group_check=True)
            nc.scalar.activation(wt[:], lp[:], ACTF.Copy)
            mx = sm.tile([P, E], F32, tag="mx")
            nc.vector.tensor_reduce(mx[:], wt[:], axis=AX.X, op=ALU.max)
            nc.vector.tensor_sub(wt[:], wt[:], mx[:, :, None].broadcast_to([P, E, E]))
            nc.scalar.activation(wt[:], wt[:], ACTF.Exp)
            sume = sm.tile([P, E], F32, tag="sume")
            nc.vector.tensor_reduce(sume[:], wt[:], axis=AX.X, op=ALU.add)
            rec = sm.tile([P, E], F32, tag="rec")
            nc.vector.reciprocal(rec[:], sume[:])
            nc.vector.tensor_mul(wt[:], wt[:], rec[:, :, None].broadcast_to([P, E, E]))

            # transpose wt -> w_all [8(e), 1024], s[e, j] via selector matmul
            wps = psb.tile([E, E, P], F32, tag="pbig")
            sps = ps.tile([E, K], F32, tag="gen")
            for pc in range(E):
                nc.tensor.transpose(wps[:, pc, :], wt[:, pc, :], ident[:])
                nc.tensor.matmul(sps[:, pc * 4:(pc + 1) * 4], wt[:, pc, :],
                                 w['sel'][:], start=True, stop=True)
            w_all = act.tile([E, E * P], BF16, tag="w_all")
            nc.scalar.activation(w_all[:], wps[:].rearrange("e a b -> e (a b)"),
                                 ACTF.Copy)
            nc.sync.dma_start(out=w_dram[t], in_=w_all[:])
            s_sb = sm.tile([E, K], F32, tag="s_sb")
            nc.scalar.activation(s_sb[:], sps[:], ACTF.Copy)
            if dbg_out is not None and t == 0:
                dw = dbgp.tile([E, E * P], F32, tag="dbg_dw")
                nc.scalar.activation(dw[:], wps[:].rearrange("e a b -> e (a b)"),
                                     ACTF.Copy)
                nc.sync.dma_start(out=dbg_out['dbg_w'][:], in_=dw[:])
                dr1 = dbgp.tile([P, C, K, K], F32, tag="dbg_r1t")
                nc.scalar.activation(dr1[:], r1[:], ACTF.Copy)
                nc.sync.dma_start(out=dbg_out['dbg_r1'][:], in_=dr1[:])

            # all-expert broadcast of w rows across 128 partitions (one DMA)
            wbca = wpool.tile([P, E, E * P], BF16, tag="wbca")
            wsrc = w_dram[t].rearrange("e q -> (e q)")
            wsrc = bass.AP(tensor=wsrc.tensor, offset=wsrc.offset,
                           ap=[[0, P]] + list(wsrc.ap))
            nc.sync.dma_start(out=wbca[:], in_=wsrc)

            # ---------- delta_self ----------
            xfs = sm.tile([P, C, K], F32, tag="xfs")
            layer_norm(h_f, w['fs_g'], w['fs_bb'], C, xfs)
            xu = act.tile([P, 2 * C, K], F32, tag="xu")
            mlp(xfs, w['fs_w1'], w['fs_b1r'], w['fs_w2'], w['fs_b2r'],
                xu[:, 0:C, :])

            # ---------- experts ----------
            dpsa = psd.tile([P, C, K], F32, tag="delta")
            dps = [dpsa[:, 0, :], dpsa[:, 1, :]]
            nc.vector.memset(dpsa[:], 0.0)
            prev_g2 = [None]
            for e in range(E):
                # token-major A^T/B^T: lhsT = h chunk, rhs = expert weights
                abp = ps.tile([K, 2 * D], F32, tag="gen")
                for kc in range(C):
                    nc.tensor.matmul(abp[:], h_b[:, kc, :],
                                     w['ex_wsij'][:, kc, e, :],
                                     start=(kc == 0), stop=False)
                # b1 folded into the B rows: each pair column picks one B row
                nc.tensor.matmul(abp[:], ones_tok_bf[:], w['ex_b1z'][0:1, e, :],
                                 start=False, stop=True)
                ab = sm.tile([2 * K, C, P], BF16, tag="ab")
                nc.scalar.activation(ab[0:K], abp[:, 0:D].rearrange(
                    "k (c p) -> k c p", c=C), ACTF.Copy)
                nc.scalar.activation(ab[K:2 * K], abp[:, D:2 * D].rearrange(
                    "k (c p) -> k c p", c=C), ACTF.Copy)

                prod = big.tile([P, C, K, K], BF16, tag="exprod")
                wv_ = wbca[:, e, :].rearrange("p (b c) -> p b c", b=K)
                for mc in range(C):
                    # pre[d,(j,i)] = A[d,i] + B[d,j] + b1[d] via selector matmul
                    pp_ = psb.tile([P, K, K], F32, tag="pbig")
                    for nh in range(2):
                        nc.tensor.matmul(pp_[:, nh * 16:(nh + 1) * 16, :],
                                         ab[:, mc, :],
                                         sel2[:, nh * 512:(nh + 1) * 512],
                                         start=True, stop=True)
                    if e % 3 == 0:
                        # fused relu+w-mul on DVE: prod = max(pre,0)*w
                        nc.vector.scalar_tensor_tensor(
                            out=prod[:, mc], in0=pp_[:], scalar=0.0, in1=wv_,
                            op0=ALU.max, op1=ALU.mult)
                    else:
                        h1m = big.tile([P, K, K], BF16, tag="exh1")
                        nc.scalar.activation(h1m[:], pp_[:], ACTF.Relu)
                        nc.vector.tensor_mul(prod[:, mc], h1m[:], wv_)
                pr4 = big.tile([P, C, K, NFOLD], BF16, tag="expr4")
                nc.vector.tensor_add(pr4[:], prod[:, :, :, 0:NFOLD],
                                     prod[:, :, :, NFOLD:2 * NFOLD])

                for kc in range(C):
                    for mc in range(C):
                        for i in range(NFOLD):
                            mm = nc.tensor.matmul(
                                dps[mc],
                                w['ex_w2'][:, kc, e, mc * P:(mc + 1) * P],
                                pr4[:, kc, :, i],
                                start=False, stop=False,
                                skip_group_check=True)
                            if prev_g2[0] is not None:
                                add_dep_helper(mm.ins, prev_g2[0], sync=False,
                                               reason="g2-weight-run order")
                            prev_g2[0] = mm.ins
                if dbg_out is not None and t == 0 and e == 0:
                    dpr = dbgp.tile([P, C, K, K], F32, tag="dbg_prod_t")
                    nc.scalar.activation(dpr[:], prod[:], ACTF.Copy)
                    nc.sync.dma_start(out=dbg_out['dbg_prod'][:], in_=dpr[:])
                    dd0 = dbgp.tile([P, C, K], F32, tag="dbg_d0_t")
                    nc.scalar.activation(dd0[:, 0, :], dps[0], ACTF.Copy)
                    nc.scalar.activation(dd0[:, 1, :], dps[1], ACTF.Copy)
                    nc.sync.dma_start(out=dbg_out['dbg_d0'][:], in_=dd0[:])
            for mc in range(C):
                nc.tensor.matmul(dps[mc], w['ex_b2t'][:, mc * P:(mc + 1) * P],
                                 s_sb[:], start=False, stop=True,
                                 skip_group_check=True)

            # ---------- update ----------
            nc.scalar.activation(xu[:, C, :], dps[0], ACTF.Copy)
            nc.scalar.activation(xu[:, C + 1, :], dps[1], ACTF.Copy)
            if dbg_out is not None and t == 0:
                nc.sync.dma_start(out=dbg_out['dbg_xu'][:], in_=xu[:])
            xln = sm.tile([P, 2 * C, K], F32, tag="xln")
            layer_norm(xu, w['up_g'], w['up_bb'], 2 * C, xln)
            hnew = act.tile([P, C, K], F32, tag="hnew")
            mlp(xln, w['up_w1'], w['up_b1r'], w['up_w2'], w['up_b2r'], hnew,
                cin=2 * C)

            # ---------- decode ----------
            hd = ps.tile([P, C, K], F32, tag="gen")
            for mc in range(C):
                for kc in range(C):
                    nc.tensor.matmul(hd[:, mc, :],
                                     w['dec_w1'][:, kc, mc * P:(mc + 1) * P],
                                     hnew[:, kc, :], start=(kc == 0), stop=False)
                bias_fold(hd[:, mc, :], w['dec_b1r'], mc, stop=True)
            hds = sm.tile([P, C, K], F32, tag="dec_h")
            nc.scalar.activation(hds[:], hd[:], ACTF.Relu)
            pp = ps.tile([DS, K], F32, tag="gen")
            for kc in range(C):
                nc.tensor.matmul(pp[:], w['dec_w2'][:, kc, :], hds[:, kc, :],
                                 start=(kc == 0), stop=False)
            nc.tensor.matmul(pp[:], w['dec_b2r'][:], ones_tok[:],
                             start=False, stop=True)
            pred_sb = act.tile([DS, K], F32, tag="pred_sb")
            nc.scalar.activation(pred_sb[:], pp[:], ACTF.Copy)
            nc.sync.dma_start(out=pred_out[t].rearrange("k s -> s k"),
                              in_=pred_sb[:])

            if t < n_steps - 1:
                encode(pred_sb)


# --------------------------------------------------------------------------
# host wrapper
# --------------------------------------------------------------------------

_PROG_CACHE = {}


def _get_program(n_steps):
    if n_steps not in _PROG_CACHE:
        _PROG_CACHE[n_steps] = build_program(n_steps)
    return _PROG_CACHE[n_steps]


def kernel(gt_states, params, rollout_steps):
    from concourse.bass_utils import run_bass_kernel_spmd

    gt = np.asarray(gt_states, np.float32)
    B, T, K_, Ds_ = gt.shape
    n_steps = min(T - 1, int(rollout_steps))
    nc = _get_program(n_steps)

    shared = prep_params(params)
    in_maps = []
    for b in range(B):
        m = dict(shared)
        m['x0t'] = np.ascontiguousarray(gt[b, 0].T)   # [8, 32]
        in_maps.append(m)

    res = run_bass_kernel_spmd(nc, in_maps, core_ids=list(range(B)))
    pred = np.stack([res.results[b]['pred'] for b in range(B)], 0)
    target = gt[:, 1:n_steps + 1]
    return pred.astype(np.float32), target


# revision 43
# speedup vs baseline: 1.0090x; 1.0090x over previous
"""Trainium2 Bass kernel for nn_MoEPairwise (MoE pairwise routing rollout).

Strategy
--------
Data-parallel over batch B=8 == 8 NeuronCores; zero collectives. Each core
runs the full 2-step rollout for one batch element (K=32 agents, D=256,
E=8 experts).

Math factorization (validated vs reference in numpy + CoreSim):
  - pair_in = [si, sj] so expert GEMM1 factors: h1_e = relu(A_e[.,i] + B_e[.,j] + b1)
    with A_e = W1si_e^T h, B_e = W1sj_e^T h  (GEMMs over 32 tokens, not 1024 pairs).
    A^T/B^T are computed token-major (lhsT = h) and the (i,j) broadcast-add runs
    on the PE as [A^T;B^T]^T @ SEL2 with a constant 0/1 selector, writing the
    1024-wide pre-activation straight into PSUM. b1 folds into the relu-evict
    bias (per-partition), w-multiply is a dense bf16 DVE op against a
    DMA-broadcast weight plane.
  - softmax weights w >= 0, so the weighted reduction folds over the pair grid:
    delta_inter = sum_e W2_e^T (sum_i w_e(j,i) * h1_e[:, (j,i)]) + b2^T s.
    i is folded 32->8 by two dense DVE adds, then contracted on the PE with
    8 accumulating matmuls per (e,kc,mc) into per-mc PSUM tiles (PSUM
    accumulation groups must not share banks).
  - router GEMM1 factors the si/sj/si-sj rows the same way; only the si*sj
    term needs a real GEMM over the 1024-pair grid.

Layouts: feature-major activations [128 partitions = d % 128, chunk, token].
Pair-grid columns ordered (j outer, i inner). bf16 for the wide expert/router
path (PE fp32 is 4x slower; DVE tensor ops 2x faster in bf16), fp32 for the
narrow backbone (LN, small MLPs) and all PSUM accumulation.
"""

import numpy as np
import ml_dtypes

import concourse.bass as bass
import concourse.mybir as mybir
import concourse.tile as tile
from concourse import bacc
from concourse.masks import make_identity
from concourse.tile_rust import add_dep_helper

F32 = mybir.dt.float32
BF16 = mybir.dt.bfloat16
AX = mybir.AxisListType
ALU = mybir.AluOpType
ACTF = mybir.ActivationFunctionType

K = 32          # agents (tokens per batch)
D = 256         # hidden
E = 8           # experts
DS = 8          # state dim
P = 128         # partitions
C = D // P      # feature chunks (2)
EPS = 1e-5
NFOLD = 16      # folded i-groups entering the G2 contraction

BF = np.dtype(ml_dtypes.bfloat16)

# packed-constant layouts: name -> (view_shape, ncols) appended in order.
# pack tiles are [npart, total_cols]; views slice columns and rearrange.
PACK_F32A = [         # [128, .] fp32, needed early (enc/router/fs)
    ('enc_w2', (P, C, D)), ('rt_wa', (P, C, D)), ('rt_wb', (P, C, D)),
    ('fs_w1', (P, C, D)), ('fs_w2', (P, C, D)),
    ('fs_g', (P, C)), ('fs_bb', (P, C)), ('sel', (P, 4)),
]
PACK_F32B = [         # [128, .] fp32, needed late (experts/update/dec)
    ('up_w1', (P, 2 * C, D)), ('up_w2', (P, C, D)),
    ('dec_w1', (P, C, D)), ('dec_w2', (P, C, DS)),
    ('up_g', (P, 2 * C)), ('up_bb', (P, 2 * C)),
    ('ex_b1', (P, C, E)),
]
PACK_RT16 = [         # [128, .] bf16, router
    ('rt_wd', (P, C, D)), ('rt_w2', (P, C, E)),
]
PACK_EX16 = [         # [128, .] bf16, experts
    ('ex_wsij', (P, C, E, 2 * D)), ('ex_w2', (P, C, E, D)),
]
PACK_P8 = [           # [8, .] fp32
    ('enc_w1', (DS, D)), ('ex_b2t', (E, D)),
]
PACK_ROWB = [         # [1, .] bf16
    ('ex_b1z', (1, E, 2 * D)),
]
PACK_ROW = [          # [1, .] fp32
    ('enc_b1r', (1, D)), ('enc_b2r', (1, D)),
    ('fs_b1r', (1, D)), ('fs_b2r', (1, D)),
    ('rt_b1r', (1, D)), ('rt_b2r', (1, E)),
    ('up_b1r', (1, D)), ('up_b2r', (1, D)),
    ('dec_b1r', (1, D)), ('dec_b2r', (1, DS)),
]


def _cols(shape):
    n = 1
    for s in shape[1:]:
        n *= s
    return n


# --------------------------------------------------------------------------
# host-side parameter preprocessing
# --------------------------------------------------------------------------

def prep_params(params):
    p = {k: np.asarray(v, np.float32) for k, v in params.items()}
    t = {}

    def chunked(x, cin=None):  # [d_in, d_out] -> [128, cin, d_out]
        d_in = x.shape[0]
        cin = d_in // P
        return x.reshape(cin, P, -1).transpose(1, 0, 2)

    t['enc_w1'] = p['enc_W1']
    t['enc_w2'] = chunked(p['enc_W2'])
    t['fs_w1'] = chunked(p['fs_W1'])
    t['fs_w2'] = chunked(p['fs_W2'])
    Wa = p['rt_W1'][0:D] + p['rt_W1'][2 * D:3 * D]
    Wb = p['rt_W1'][D:2 * D] - p['rt_W1'][2 * D:3 * D]
    t['rt_wa'] = chunked(Wa)
    t['rt_wb'] = chunked(Wb)
    t['rt_wd'] = chunked(p['rt_W1'][3 * D:4 * D])
    t['rt_w2'] = chunked(p['rt_W2'])
    wsi = p['ex_W1'][:, 0:D, :].reshape(E, C, P, D).transpose(2, 1, 0, 3)
    wsj = p['ex_W1'][:, D:2 * D, :].reshape(E, C, P, D).transpose(2, 1, 0, 3)
    t['ex_wsij'] = np.concatenate([wsi, wsj], axis=-1)   # [128, C, E, 512]
    t['ex_w2'] = p['ex_W2'].reshape(E, C, P, D).transpose(2, 1, 0, 3)
    t['ex_b1'] = p['ex_b1'].T.reshape(C, P, E).transpose(1, 0, 2)   # [128, C, E]
    t['ex_b1z'] = np.concatenate([np.zeros_like(p['ex_b1']), p['ex_b1']],
                                 axis=1)[None]                      # [1, E, 512]
    t['ex_b2t'] = p['ex_b2']
    t['up_w1'] = chunked(p['up_W1'])
    t['up_w2'] = chunked(p['up_W2'])
    t['dec_w1'] = chunked(p['dec_W1'])
    t['dec_w2'] = chunked(p['dec_W2'])
    t['fs_g'] = p['fs_g'].reshape(C, P).T
    t['fs_bb'] = p['fs_b'].reshape(C, P).T
    t['up_g'] = p['up_g'].reshape(2 * C, P).T
    t['up_bb'] = p['up_b'].reshape(2 * C, P).T
    for n_, b_ in [('enc_b1r', 'enc_b1'), ('enc_b2r', 'enc_b2'),
                   ('fs_b1r', 'fs_b1'), ('fs_b2r', 'fs_b2'),
                   ('rt_b1r', 'rt_b1'), ('rt_b2r', 'rt_b2'),
                   ('up_b1r', 'up_b1'), ('up_b2r', 'up_b2'),
                   ('dec_b1r', 'dec_b1'), ('dec_b2r', 'dec_b2')]:
        t[n_] = p[b_][None, :]
    sel = np.zeros((P, 4), np.float32)
    for jl in range(4):
        sel[jl * K:(jl + 1) * K, jl] = 1.0
    t['sel'] = sel

    def pack(layout, npart, dtype):
        cols = sum(_cols(sh) for _, sh in layout)
        buf = np.zeros((npart, cols), np.float32)
        off = 0
        for name, sh in layout:
            n = _cols(sh)
            arr = t[name]
            assert tuple(arr.shape) == tuple(sh), (name, arr.shape, sh)
            buf[:sh[0], off:off + n] = arr.reshape(sh[0], n)
            off += n
        return np.ascontiguousarray(buf.astype(dtype))

    ins = {
        'pack_f32a': pack(PACK_F32A, P, np.float32),
        'pack_f32b': pack(PACK_F32B, P, np.float32),
        'pack_rt16': pack(PACK_RT16, P, BF),
        'pack_ex16': pack(PACK_EX16, P, BF),
        'pack_p8': pack(PACK_P8, DS, np.float32),
        'pack_row': pack(PACK_ROW, 1, np.float32),
        'pack_rowb': pack(PACK_ROWB, 1, BF),
    }
    # SEL2 [64, 1024]: row k<32 selects i==k; row k>=32 selects j==k-32
    sel2 = np.zeros((2 * K, K * K), np.float32)
    for i in range(K):
        sel2[i, i::K] = 1.0          # columns (j, i) with that i
        sel2[K + i, i * K:(i + 1) * K] = 1.0   # columns with j == i
    ins['sel2'] = np.ascontiguousarray(sel2.astype(BF))
    return ins


# --------------------------------------------------------------------------
# device program
# --------------------------------------------------------------------------

def build_program(n_steps=2, dbg=False):
    nc = bacc.Bacc("TRN2", target_bir_lowering=False, debug=False)

    dr = {}
    specs = {
        'x0t': ([DS, K], F32),
        'pack_f32a': ([P, sum(_cols(s) for _, s in PACK_F32A)], F32),
        'pack_f32b': ([P, sum(_cols(s) for _, s in PACK_F32B)], F32),
        'pack_rt16': ([P, sum(_cols(s) for _, s in PACK_RT16)], BF16),
        'pack_ex16': ([P, sum(_cols(s) for _, s in PACK_EX16)], BF16),
        'pack_p8': ([DS, sum(_cols(s) for _, s in PACK_P8)], F32),
        'pack_row': ([1, sum(_cols(s) for _, s in PACK_ROW)], F32),
        'pack_rowb': ([1, sum(_cols(s) for _, s in PACK_ROWB)], BF16),
        'sel2': ([2 * K, K * K], BF16),
    }
    for name, (shape, dt) in specs.items():
        dr[name] = nc.dram_tensor(name, shape, dt, kind="ExternalInput").ap()
    pred_out = nc.dram_tensor('pred', [n_steps, K, DS], F32, kind="ExternalOutput").ap()
    w_dram = nc.dram_tensor('w_scratch', [n_steps, E, E * P], BF16, kind="Internal").ap()
    dbg_out = None
    if dbg:
        dbg_out = {
            'dbg_h0': nc.dram_tensor('dbg_h0', [P, C, K], F32, kind="ExternalOutput").ap(),
            'dbg_w': nc.dram_tensor('dbg_w', [E, E * P], F32, kind="ExternalOutput").ap(),
            'dbg_xu': nc.dram_tensor('dbg_xu', [P, 2 * C, K], F32, kind="ExternalOutput").ap(),
            'dbg_r1': nc.dram_tensor('dbg_r1', [P, C, K, K], F32, kind="ExternalOutput").ap(),
            'dbg_prod': nc.dram_tensor('dbg_prod', [P, C, K, K], F32, kind="ExternalOutput").ap(),
            'dbg_d0': nc.dram_tensor('dbg_d0', [P, C, K], F32, kind="ExternalOutput").ap(),
        }

    with tile.TileContext(nc) as tc:
        _emit(nc, tc, dr, pred_out, n_steps, dbg_out, w_dram)
    nc.compile()
    return nc


def _emit(nc, tc, dr, pred_out, n_steps, dbg_out=None, w_dram=None):
    from contextlib import ExitStack
    ctx = ExitStack()
    with ctx:
        const = ctx.enter_context(tc.tile_pool(name="const", bufs=1))
        act = ctx.enter_context(tc.tile_pool(name="act", bufs=2))
        big = ctx.enter_context(tc.tile_pool(name="big", bufs=4))
        wpool = ctx.enter_context(tc.tile_pool(name="wpool", bufs=2))
        sm = ctx.enter_context(tc.tile_pool(name="sm", bufs=4))
        dbgp = ctx.enter_context(tc.tile_pool(name="dbgp", bufs=1))
        ps = ctx.enter_context(tc.tile_pool(name="ps", bufs=3, space="PSUM"))
        psb = ctx.enter_context(tc.tile_pool(name="psb", bufs=2, space="PSUM"))
        psd = ctx.enter_context(tc.tile_pool(name="psd", bufs=1, space="PSUM"))

        # ---- load packed constants, build per-name views
        packs = {}
        for pname in ('x0t', 'pack_p8', 'pack_f32a', 'pack_rt16', 'pack_row',
                      'sel2', 'pack_ex16', 'pack_f32b', 'pack_rowb'):
            ap = dr[pname]
            t_ = const.tile(ap.shape, ap.dtype, tag=pname)
            nc.sync.dma_start(out=t_[:], in_=ap[:])
            packs[pname] = t_

        w = {}
        for pname, layout in [('pack_f32a', PACK_F32A), ('pack_f32b', PACK_F32B),
                              ('pack_rt16', PACK_RT16), ('pack_ex16', PACK_EX16),
                              ('pack_p8', PACK_P8), ('pack_row', PACK_ROW),
                              ('pack_rowb', PACK_ROWB)]:
            off = 0
            for name, sh in layout:
                n = _cols(sh)
                view = packs[pname][:sh[0], off:off + n]
                if len(sh) == 3:
                    view = view.rearrange("p (a b) -> p a b", a=sh[1])
                elif len(sh) == 4:
                    view = view.rearrange("p (a b c) -> p a b c", a=sh[1], b=sh[2])
                w[name] = view
                off += n
        sel2 = packs['sel2']

        ones_col = const.tile([P, 1], F32)      # partition-sum matmuls
        nc.vector.memset(ones_col[:], 1.0)
        ones_row = const.tile([1, P], F32)      # partition-broadcast matmuls
        nc.vector.memset(ones_row[:], 1.0)
        ones_tok = const.tile([1, K], F32)      # bias-fold matmuls
        nc.vector.memset(ones_tok[:], 1.0)
        ones_tok_bf = const.tile([1, K], BF16)
        nc.vector.memset(ones_tok_bf[:], 1.0)
        ident = const.tile([P, P], F32)
        make_identity(nc, ident)
        eps_t = const.tile([1, 1], F32)
        nc.vector.memset(eps_t[:], EPS)

        h_f = act.tile([P, C, K], F32, tag="h_f")
        h_b = act.tile([P, C, K], BF16, tag="h_b")

        # ---- helpers ------------------------------------------------------
        def bias_fold(out_ps_mc, brow, mc, width=P, stop=False):
            nc.tensor.matmul(out_ps_mc, brow[:, mc * P:mc * P + width],
                             ones_tok[:], start=False, stop=stop)

        def mlp(x, win1, b1r, win2, b2r, out_sb, cin=C):
            hp = ps.tile([P, C, K], F32, tag="gen")
            for mc in range(C):
                for kc in range(cin):
                    nc.tensor.matmul(hp[:, mc, :],
                                     win1[:, kc, mc * P:(mc + 1) * P], x[:, kc, :],
                                     start=(kc == 0), stop=False)
                bias_fold(hp[:, mc, :], b1r, mc, stop=True)
            hs = sm.tile([P, C, K], F32, tag="mlp_h")
            nc.scalar.activation(hs[:], hp[:], ACTF.Relu)
            op = ps.tile([P, C, K], F32, tag="gen")
            for mc in range(C):
                for kc in range(C):
                    nc.tensor.matmul(op[:, mc, :],
                                     win2[:, kc, mc * P:(mc + 1) * P], hs[:, kc, :],
                                     start=(kc == 0), stop=False)
                bias_fold(op[:, mc, :], b2r, mc, stop=True)
            nc.scalar.activation(out_sb, op[:], ACTF.Copy)
            return out_sb

        def layer_norm(x, g, bb, cin, out):
            sq = sm.tile([P, cin, K], F32, tag="ln_sq")
            nc.vector.tensor_mul(sq[:], x[:], x[:])
            mp = ps.tile([1, K], F32, tag="gen")
            qp = ps.tile([1, K], F32, tag="gen")
            for kc in range(cin):
                nc.tensor.matmul(mp[:], ones_col[:], x[:, kc, :],
                                 start=(kc == 0), stop=(kc == cin - 1))
            for kc in range(cin):
                nc.tensor.matmul(qp[:], ones_col[:], sq[:, kc, :],
                                 start=(kc == 0), stop=(kc == cin - 1))
            st = sm.tile([1, 2, K], F32, tag="ln_st")
            dinv = 1.0 / (cin * P)
            nc.vector.tensor_scalar(out=st[:, 0, :], in0=mp[:], scalar1=dinv,
                                    scalar2=None, op0=ALU.mult)
            m2 = sm.tile([1, K], F32, tag="ln_m2")
            nc.vector.tensor_scalar(out=m2[:], in0=qp[:], scalar1=dinv,
                                    scalar2=None, op0=ALU.mult)
            var = sm.tile([1, K], F32, tag="ln_var")
            nc.vector.tensor_mul(var[:], st[:, 0, :], st[:, 0, :])
            nc.vector.tensor_sub(var[:], m2[:], var[:])
            std = sm.tile([1, K], F32, tag="ln_std")
            nc.scalar.activation(std[:], var[:], ACTF.Sqrt, bias=eps_t[:])
            nc.vector.reciprocal(st[:, 1, :], std[:])
            bp = ps.tile([P, 2, K], F32, tag="gen")
            nc.tensor.matmul(bp[:, 0, :], ones_row[:], st[:, 0, :], start=True, stop=True)
            nc.tensor.matmul(bp[:, 1, :], ones_row[:], st[:, 1, :], start=True, stop=True)
            t1 = sm.tile([P, cin, K], F32, tag="ln_t1")
            mb = bp[:, 0:1, :].broadcast_to([P, cin, K])
            rb = bp[:, 1:2, :].broadcast_to([P, cin, K])
            nc.vector.tensor_sub(t1[:], x[:], mb)
            nc.vector.tensor_mul(t1[:], t1[:], rb)
            for c_ in range(cin):
                nc.vector.tensor_scalar(out=out[:, c_, :], in0=t1[:, c_, :],
                                        scalar1=g[:, c_:c_ + 1],
                                        scalar2=bb[:, c_:c_ + 1],
                                        op0=ALU.mult, op1=ALU.add)
            return out

        def encode(x_sb):
            hp = ps.tile([P, C, K], F32, tag="gen")
            for mc in range(C):
                nc.tensor.matmul(hp[:, mc, :], w['enc_w1'][:, mc * P:(mc + 1) * P],
                                 x_sb[:], start=True, stop=False)
                bias_fold(hp[:, mc, :], w['enc_b1r'], mc, stop=True)
            hs = sm.tile([P, C, K], F32, tag="enc_h")
            nc.scalar.activation(hs[:], hp[:], ACTF.Relu)
            op = ps.tile([P, C, K], F32, tag="gen")
            for mc in range(C):
                for kc in range(C):
                    nc.tensor.matmul(op[:, mc, :],
                                     w['enc_w2'][:, kc, mc * P:(mc + 1) * P],
                                     hs[:, kc, :], start=(kc == 0), stop=False)
                bias_fold(op[:, mc, :], w['enc_b2r'], mc, stop=True)
            nc.scalar.activation(h_f[:], op[:], ACTF.Copy)
            nc.scalar.activation(h_b[:], op[:], ACTF.Copy)

        # ---- initial encode
        x0 = packs['x0t']
        encode(x0)
        if dbg_out is not None:
            nc.sync.dma_start(out=dbg_out['dbg_h0'][:], in_=h_f[:])

        # ---- rollout steps
        for t in range(n_steps):
            # ---------- router ----------
            mulij = big.tile([P, C, K, K], BF16, tag="mulij")
            for kc in range(C):
                nc.vector.tensor_mul(
                    mulij[:, kc],
                    h_b[:, kc, None, :].broadcast_to([P, K, K]),
                    h_b[:, kc, :, None].broadcast_to([P, K, K]))

            arp = ps.tile([P, C, K], F32, tag="gen")
            brp = ps.tile([P, C, K], F32, tag="gen")
            for mc in range(C):
                for kc in range(C):
                    nc.tensor.matmul(arp[:, mc, :],
                                     w['rt_wa'][:, kc, mc * P:(mc + 1) * P],
                                     h_f[:, kc, :], start=(kc == 0), stop=False)
                bias_fold(arp[:, mc, :], w['rt_b1r'], mc, stop=True)
                for kc in range(C):
                    nc.tensor.matmul(brp[:, mc, :],
                                     w['rt_wb'][:, kc, mc * P:(mc + 1) * P],
                                     h_f[:, kc, :], start=(kc == 0),
                                     stop=(kc == C - 1))
            ar = sm.tile([P, C, K], BF16, tag="ar")
            br = sm.tile([P, C, K], BF16, tag="br")
            nc.scalar.activation(ar[:], arp[:], ACTF.Copy)
            nc.scalar.activation(br[:], brp[:], ACTF.Copy)

            r1 = big.tile([P, C, K, K], BF16, tag="r1")
            for mc in range(C):
                cp = psb.tile([P, K, K], F32, tag="pbig")
                for kc in range(C):
                    for nh in range(2):
                        nc.tensor.matmul(
                            cp[:, nh * 16:(nh + 1) * 16, :],
                            w['rt_wd'][:, kc, mc * P:(mc + 1) * P],
                            mulij[:, kc, nh * 16:(nh + 1) * 16, :],
                            start=(kc == 0), stop=(kc == C - 1))
                pre = big.tile([P, K, K], BF16, tag="rpre")
                nc.gpsimd.tensor_tensor(
                    pre[:], ar[:, mc, None, :].broadcast_to([P, K, K]),
                    br[:, mc, :, None].broadcast_to([P, K, K]), op=ALU.add)
                nc.vector.tensor_add(pre[:], pre[:], cp[:])
                nc.scalar.activation(r1[:, mc, :, :], pre[:], ACTF.Relu)

            # logits_T + softmax over free dim e, per 128-pair chunk
            wt = sm.tile([P, E, E], F32, tag="wt")
            lp = ps.tile([P, E, E], F32, tag="gen")
            nc.vector.memset(lp[:], 0.0)
            for pc in range(E):
                for kc in range(C):
                    nc.tensor.matmul(lp[:, pc, :], r1[:, kc, :, :].rearrange(
                        "p a b -> p (a b)")[:, pc * P:(pc + 1) * P],
                        w['rt_w2'][:, kc, :], start=False,
                        stop=(pc == E - 1 and kc == C - 1),
                        skip_group_check=True)
                nc.tensor.matmul(lp[:, pc, :], ones_row[:], w['rt_b2r'][:],
                                 start=False, stop=False,
                                 skip_group_check=True)
            nc.scalar.activation(wt[:], lp[:], ACTF.Copy)
            mx = sm.tile([P, E], F32, tag="mx")
            nc.vector.tensor_reduce(mx[:], wt[:], axis=AX.X, op=ALU.max)
            nc.vector.tensor_sub(wt[:], wt[:], mx[:, :, None].broadcast_to([P, E, E]))
            nc.scalar.activation(wt[:], wt[:], ACTF.Exp)
            sume = sm.tile([P, E], F32, tag="sume")
            nc.vector.tensor_reduce(sume[:], wt[:], axis=AX.X, op=ALU.add)
            rec = sm.tile([P, E], F32, tag="rec")
            nc.vector.reciprocal(rec[:], sume[:])
            nc.vector.tensor_mul(wt[:], wt[:], rec[:, :, None].broadcast_to([P, E, E]))

            # transpose wt -> w_all [8(e), 1024], s[e, j] via selector matmul
            wps = psb.tile([E, E, P], F32, tag="pbig")
            sps = ps.tile([E, K], F32, tag="gen")
            for pc in range(E):
                nc.tensor.transpose(wps[:, pc, :], wt[:, pc, :], ident[:])
                nc.tensor.matmul(sps[:, pc * 4:(pc + 1) * 4], wt[:, pc, :],
                                 w['sel'][:], start=True, stop=True)
            w_all = act.tile([E, E * P], BF16, tag="w_all")
            nc.scalar.activation(w_all[:], wps[:].rearrange("e a b -> e (a b)"),
                                 ACTF.Copy)
            nc.sync.dma_start(out=w_dram[t], in_=w_all[:])
            s_sb = sm.tile([E, K], F32, tag="s_sb")
            nc.scalar.activation(s_sb[:], sps[:], ACTF.Copy)
            if dbg_out is not None and t == 0:
                dw = dbgp.tile([E, E * P], F32, tag="dbg_dw")
                nc.scalar.activation(dw[:], wps[:].rearrange("e a b -> e (a b)"),
                                     ACTF.Copy)
                nc.sync.dma_start(out=dbg_out['dbg_w'][:], in_=dw[:])
                dr1 = dbgp.tile([P, C, K, K], F32, tag="dbg_r1t")
                nc.scalar.activation(dr1[:], r1[:], ACTF.Copy)
                nc.sync.dma_start(out=dbg_out['dbg_r1'][:], in_=dr1[:])

            # all-expert broadcast of w rows across 128 partitions (one DMA)
            wbca = wpool.tile([P, E, E * P], BF16, tag="wbca")
            wsrc = w_dram[t].rearrange("e q -> (e q)")
            wsrc = bass.AP(tensor=wsrc.tensor, offset=wsrc.offset,
                           ap=[[0, P]] + list(wsrc.ap))
            nc.sync.dma_start(out=wbca[:], in_=wsrc)

            # ---------- delta_self ----------
            xfs = sm.tile([P, C, K], F32, tag="xfs")
            layer_norm(h_f, w['fs_g'], w['fs_bb'], C, xfs)
            xu = act.tile([P, 2 * C, K], F32, tag="xu")
            mlp(xfs, w['fs_w1'], w['fs_b1r'], w['fs_w2'], w['fs_b2r'],
                xu[:, 0:C, :])

            # ---------- experts ----------
            dpsa = psd.tile([P, C, K], F32, tag="delta")
            dps = [dpsa[:, 0, :], dpsa[:, 1, :]]
            nc.vector.memset(dpsa[:], 0.0)
            prev_g2 = [None]
            for e in range(E):
                # token-major A^T/B^T: lhsT = h chunk, rhs = expert weights
                abp = ps.tile([K, 2 * D], F32, tag="gen")
                for kc in range(C):
                    nc.tensor.matmul(abp[:], h_b[:, kc, :],
                                     w['ex_wsij'][:, kc, e, :],
                                     start=(kc == 0), stop=False)
                # b1 folded into the B rows: each pair column picks one B row
                nc.tensor.matmul(abp[:], ones_tok_bf[:], w['ex_b1z'][0:1, e, :],
                                 start=False, stop=True)
                ab = sm.tile([2 * K, C, P], BF16, tag="ab")
                nc.scalar.activation(ab[0:K], abp[:, 0:D].rearrange(
                    "k (c p) -> k c p", c=C), ACTF.Copy)
                nc.scalar.activation(ab[K:2 * K], abp[:, D:2 * D].rearrange(
                    "k (c p) -> k c p", c=C), ACTF.Copy)

                prod = big.tile([P, C, K, K], BF16, tag="exprod")
                wv_ = wbca[:, e, :].rearrange("p (b c) -> p b c", b=K)
                for mc in range(C):
                    # pre[d,(j,i)] = A[d,i] + B[d,j] + b1[d] via selector matmul
                    pp_ = psb.tile([P, K, K], F32, tag="pbig")
                    for nh in range(2):
                        nc.tensor.matmul(pp_[:, nh * 16:(nh + 1) * 16, :],
                                         ab[:, mc, :],
                                         sel2[:, nh * 512:(nh + 1) * 512],
                                         start=True, stop=True)
                    if e % 3 == 0:
                        # fused relu+w-mul on DVE: prod = max(pre,0)*w
                        nc.vector.scalar_tensor_tensor(
                            out=prod[:, mc], in0=pp_[:], scalar=0.0, in1=wv_,
                            op0=ALU.max, op1=ALU.mult)
                    else:
                        h1m = big.tile([P, K, K], BF16, tag="exh1")
                        nc.scalar.activation(h1m[:], pp_[:], ACTF.Relu)
                        nc.vector.tensor_mul(prod[:, mc], h1m[:], wv_)
                pr4 = big.tile([P, C, K, NFOLD], BF16, tag="expr4")
                nc.vector.tensor_add(pr4[:], prod[:, :, :, 0:NFOLD],
                                     prod[:, :, :, NFOLD:2 * NFOLD])

                for kc in range(C):
                    for mc in range(C):
                        for i in range(NFOLD):
                            mm = nc.tensor.matmul(
                                dps[mc],
                                w['ex_w2'][:, kc, e, mc * P:(mc + 1) * P],
                                pr4[:, kc, :, i],
                                start=False, stop=False,
                                skip_group_check=True)
                            if prev_g2[0] is not None:
                                add_dep_helper(mm.ins, prev_g2[0], sync=False,
                                               reason="g2-weight-run order")
                            prev_g2[0] = mm.ins
                if dbg_out is not None and t == 0 and e == 0:
                    dpr = dbgp.tile([P, C, K, K], F32, tag="dbg_prod_t")
                    nc.scalar.activation(dpr[:], prod[:], ACTF.Copy)
                    nc.sync.dma_start(out=dbg_out['dbg_prod'][:], in_=dpr[:])
                    dd0 = dbgp.tile([P, C, K], F32, tag="dbg_d0_t")
                    nc.scalar.activation(dd0[:, 0, :], dps[0], ACTF.Copy)
                    nc.scalar.activation(dd0[:, 1, :], dps[1], ACTF.Copy)
                    nc.sync.dma_start(out=dbg_out['dbg_d0'][:], in_=dd0[:])
            for mc in range(C):
                nc.tensor.matmul(dps[mc], w['ex_b2t'][:, mc * P:(mc + 1) * P],
                                 s_sb[:], start=False, stop=True,
                                 skip_group_check=True)

            # ---------- update ----------
            nc.scalar.activation(xu[:, C, :], dps[0], ACTF.Copy)
            nc.scalar.activation(xu[:, C + 1, :], dps[1], ACTF.Copy)
            if dbg_out is not None and t == 0:
                nc.sync.dma_start(out=dbg_out['dbg_xu'][:], in_=xu[:])
            xln = sm.tile([P, 2 * C, K], F32, tag="xln")
            layer_norm(xu, w['up_g'], w['up_bb'], 2 * C, xln)
            hnew = act.tile([P, C, K], F32, tag="hnew")
            mlp(xln, w['up_w1'], w['up_b1r'], w['up_w2'], w['up_b2r'], hnew,
                cin=2 * C)

            # ---------- decode ----------
            hd = ps.tile([P, C, K], F32, tag="gen")
            for mc in range(C):
                for kc in range(C):
                    nc.tensor.matmul(hd[:, mc, :],
                                     w['dec_w1'][:, kc, mc * P:(mc + 1) * P],
                                     hnew[:, kc, :], start=(kc == 0), stop=False)
                bias_fold(hd[:, mc, :], w['dec_b1r'], mc, stop=True)
            hds = sm.tile([P, C, K], F32, tag="dec_h")
            nc.scalar.activation(hds[:], hd[:], ACTF.Relu)
            pp = ps.tile([DS, K], F32, tag="gen")
            for kc in range(C):
                nc.tensor.matmul(pp[:], w['dec_w2'][:, kc, :], hds[:, kc, :],
                                 start=(kc == 0), stop=False)
            nc.tensor.matmul(pp[:], w['dec_b2r'][:], ones_tok[:],
                             start=False, stop=True)
            pred_sb = act.tile([DS, K], F32, tag="pred_sb")
            nc.scalar.activation(pred_sb[:], pp[:], ACTF.Copy)
            nc.sync.dma_start(out=pred_out[t].rearrange("k s -> s k"),
                              in_=pred_sb[:])

            if t < n_steps - 1:
                encode(pred_sb)


# --------------------------------------------------------------------------
# host wrapper
# --------------------------------------------------------------------------

_PROG_CACHE = {}


def _get_program(n_steps):
    if n_steps not in _PROG_CACHE:
        _PROG_CACHE[n_steps] = build_program(n_steps)
    return _PROG_CACHE[n_steps]


def kernel(gt_states, params, rollout_steps):
    from concourse.bass_utils import run_bass_kernel_spmd

    gt = np.asarray(gt_states, np.float32)
    B, T, K_, Ds_ = gt.shape
    n_steps = min(T - 1, int(rollout_steps))
    nc = _get_program(n_steps)

    shared = prep_params(params)
    in_maps = []
    for b in range(B):
        m = dict(shared)
        m['x0t'] = np.ascontiguousarray(gt[b, 0].T)   # [8, 32]
        in_maps.append(m)

    res = run_bass_kernel_spmd(nc, in_maps, core_ids=list(range(B)))
    pred = np.stack([res.results[b]['pred'] for b in range(B)], 0)
    target = gt[:, 1:n_steps + 1]
    return pred.astype(np.float32), target


# revision 47
# speedup vs baseline: 1.2121x; 1.2012x over previous
"""Trainium2 Bass kernel for nn_MoEPairwise (MoE pairwise routing rollout).

Strategy
--------
Data-parallel over batch B=8 == 8 NeuronCores; zero collectives. Each core
runs the full 2-step rollout for one batch element (K=32 agents, D=256,
E=8 experts).

Math factorization (validated vs reference in numpy + CoreSim):
  - pair_in = [si, sj] so expert GEMM1 factors: h1_e = relu(A_e[.,i] + B_e[.,j] + b1)
    with A_e = W1si_e^T h, B_e = W1sj_e^T h  (GEMMs over 32 tokens, not 1024 pairs).
    A^T/B^T are computed token-major (lhsT = h) and the (i,j) broadcast-add runs
    on the PE as [A^T;B^T]^T @ SEL2 with a constant 0/1 selector, writing the
    1024-wide pre-activation straight into PSUM. b1 folds into the relu-evict
    bias (per-partition), w-multiply is a dense bf16 DVE op against a
    DMA-broadcast weight plane.
  - softmax weights w >= 0, so the weighted reduction folds over the pair grid:
    delta_inter = sum_e W2_e^T (sum_i w_e(j,i) * h1_e[:, (j,i)]) + b2^T s.
    i is folded 32->8 by two dense DVE adds, then contracted on the PE with
    8 accumulating matmuls per (e,kc,mc) into per-mc PSUM tiles (PSUM
    accumulation groups must not share banks).
  - router GEMM1 factors the si/sj/si-sj rows the same way; only the si*sj
    term needs a real GEMM over the 1024-pair grid.

Layouts: feature-major activations [128 partitions = d % 128, chunk, token].
Pair-grid columns ordered (j outer, i inner). bf16 for the wide expert/router
path (PE fp32 is 4x slower; DVE tensor ops 2x faster in bf16), fp32 for the
narrow backbone (LN, small MLPs) and all PSUM accumulation.
"""

import numpy as np
import ml_dtypes

import concourse.bass as bass
import concourse.mybir as mybir
import concourse.tile as tile
from concourse import bacc
from concourse.masks import make_identity
from concourse.tile_rust import add_dep_helper

F32 = mybir.dt.float32
BF16 = mybir.dt.bfloat16
AX = mybir.AxisListType
ALU = mybir.AluOpType
ACTF = mybir.ActivationFunctionType

K = 32          # agents (tokens per batch)
D = 256         # hidden
E = 8           # experts
DS = 8          # state dim
P = 128         # partitions
C = D // P      # feature chunks (2)
EPS = 1e-5
NFOLD = 4       # folded i-groups entering the G2 contraction

BF = np.dtype(ml_dtypes.bfloat16)

# packed-constant layouts: name -> (view_shape, ncols) appended in order.
# pack tiles are [npart, total_cols]; views slice columns and rearrange.
PACK_F32A = [         # [128, .] fp32, needed early (enc/router/fs)
    ('enc_w2', (P, C, D)), ('rt_wa', (P, C, D)), ('rt_wb', (P, C, D)),
    ('fs_w1', (P, C, D)), ('fs_w2', (P, C, D)),
    ('fs_g', (P, C)), ('fs_bb', (P, C)), ('sel', (P, 4)),
]
PACK_F32B = [         # [128, .] fp32, needed late (experts/update/dec)
    ('up_w1', (P, 2 * C, D)), ('up_w2', (P, C, D)),
    ('dec_w1', (P, C, D)), ('dec_w2', (P, C, DS)),
    ('up_g', (P, 2 * C)), ('up_bb', (P, 2 * C)),
    ('ex_b1', (P, C, E)),
]
PACK_RT16 = [         # [128, .] bf16, router
    ('rt_wd', (P, C, D)), ('rt_w2', (P, C, E)),
]
PACK_EX16 = [         # [128, .] bf16, experts
    ('ex_wsij', (P, C, E, 2 * D)), ('ex_w2', (P, C, E, D)),
]
PACK_P8 = [           # [8, .] fp32
    ('enc_w1', (DS, D)), ('ex_b2t', (E, D)), ('rt_b2c', (E, 1)),
]
PACK_ROWB = [         # [1, .] bf16
    ('ex_b1z', (1, E, 2 * D)),
]
PACK_ROW = [          # [1, .] fp32
    ('enc_b1r', (1, D)), ('enc_b2r', (1, D)),
    ('fs_b1r', (1, D)), ('fs_b2r', (1, D)),
    ('rt_b1r', (1, D)), ('rt_b2r', (1, E)),
    ('up_b1r', (1, D)), ('up_b2r', (1, D)),
    ('dec_b1r', (1, D)), ('dec_b2r', (1, DS)),
]


def _cols(shape):
    n = 1
    for s in shape[1:]:
        n *= s
    return n


# --------------------------------------------------------------------------
# host-side parameter preprocessing
# --------------------------------------------------------------------------

def prep_params(params):
    p = {k: np.asarray(v, np.float32) for k, v in params.items()}
    t = {}

    def chunked(x, cin=None):  # [d_in, d_out] -> [128, cin, d_out]
        d_in = x.shape[0]
        cin = d_in // P
        return x.reshape(cin, P, -1).transpose(1, 0, 2)

    t['enc_w1'] = p['enc_W1']
    t['enc_w2'] = chunked(p['enc_W2'])
    t['fs_w1'] = chunked(p['fs_W1'])
    t['fs_w2'] = chunked(p['fs_W2'])
    Wa = p['rt_W1'][0:D] + p['rt_W1'][2 * D:3 * D]
    Wb = p['rt_W1'][D:2 * D] - p['rt_W1'][2 * D:3 * D]
    t['rt_wa'] = chunked(Wa)
    t['rt_wb'] = chunked(Wb)
    t['rt_wd'] = chunked(p['rt_W1'][3 * D:4 * D])
    t['rt_w2'] = chunked(p['rt_W2'])
    wsi = p['ex_W1'][:, 0:D, :].reshape(E, C, P, D).transpose(2, 1, 0, 3)
    wsj = p['ex_W1'][:, D:2 * D, :].reshape(E, C, P, D).transpose(2, 1, 0, 3)
    t['ex_wsij'] = np.concatenate([wsi, wsj], axis=-1)   # [128, C, E, 512]
    t['ex_w2'] = p['ex_W2'].reshape(E, C, P, D).transpose(2, 1, 0, 3)
    t['ex_b1'] = p['ex_b1'].T.reshape(C, P, E).transpose(1, 0, 2)   # [128, C, E]
    t['ex_b1z'] = np.concatenate([np.zeros_like(p['ex_b1']), p['ex_b1']],
                                 axis=1)[None]                      # [1, E, 512]
    t['ex_b2t'] = p['ex_b2']
    t['rt_b2c'] = p['rt_b2'][:, None]
    t['up_w1'] = chunked(p['up_W1'])
    t['up_w2'] = chunked(p['up_W2'])
    t['dec_w1'] = chunked(p['dec_W1'])
    t['dec_w2'] = chunked(p['dec_W2'])
    t['fs_g'] = p['fs_g'].reshape(C, P).T
    t['fs_bb'] = p['fs_b'].reshape(C, P).T
    t['up_g'] = p['up_g'].reshape(2 * C, P).T
    t['up_bb'] = p['up_b'].reshape(2 * C, P).T
    for n_, b_ in [('enc_b1r', 'enc_b1'), ('enc_b2r', 'enc_b2'),
                   ('fs_b1r', 'fs_b1'), ('fs_b2r', 'fs_b2'),
                   ('rt_b1r', 'rt_b1'), ('rt_b2r', 'rt_b2'),
                   ('up_b1r', 'up_b1'), ('up_b2r', 'up_b2'),
                   ('dec_b1r', 'dec_b1'), ('dec_b2r', 'dec_b2')]:
        t[n_] = p[b_][None, :]
    sel = np.zeros((P, 4), np.float32)
    for jl in range(4):
        sel[jl * K:(jl + 1) * K, jl] = 1.0
    t['sel'] = sel

    def pack(layout, npart, dtype):
        cols = sum(_cols(sh) for _, sh in layout)
        buf = np.zeros((npart, cols), np.float32)
        off = 0
        for name, sh in layout:
            n = _cols(sh)
            arr = t[name]
            assert tuple(arr.shape) == tuple(sh), (name, arr.shape, sh)
            buf[:sh[0], off:off + n] = arr.reshape(sh[0], n)
            off += n
        return np.ascontiguousarray(buf.astype(dtype))

    ins = {
        'pack_f32a': pack(PACK_F32A, P, np.float32),
        'pack_f32b': pack(PACK_F32B, P, np.float32),
        'pack_rt16': pack(PACK_RT16, P, BF),
        'pack_ex16': pack(PACK_EX16, P, BF),
        'pack_p8': pack(PACK_P8, DS, np.float32),
        'pack_row': pack(PACK_ROW, 1, np.float32),
        'pack_rowb': pack(PACK_ROWB, 1, BF),
    }
    # SEL2 [64, 1024]: row k<32 selects i==k; row k>=32 selects j==k-32
    sel2 = np.zeros((2 * K, K * K), np.float32)
    for i in range(K):
        sel2[i, i::K] = 1.0          # columns (j, i) with that i
        sel2[K + i, i * K:(i + 1) * K] = 1.0   # columns with j == i
    ins['sel2'] = np.ascontiguousarray(sel2.astype(BF))
    return ins


# --------------------------------------------------------------------------
# device program
# --------------------------------------------------------------------------

def build_program(n_steps=2, dbg=False):
    nc = bacc.Bacc("TRN2", target_bir_lowering=False, debug=False)

    dr = {}
    specs = {
        'x0t': ([DS, K], F32),
        'pack_f32a': ([P, sum(_cols(s) for _, s in PACK_F32A)], F32),
        'pack_f32b': ([P, sum(_cols(s) for _, s in PACK_F32B)], F32),
        'pack_rt16': ([P, sum(_cols(s) for _, s in PACK_RT16)], BF16),
        'pack_ex16': ([P, sum(_cols(s) for _, s in PACK_EX16)], BF16),
        'pack_p8': ([DS, sum(_cols(s) for _, s in PACK_P8)], F32),
        'pack_row': ([1, sum(_cols(s) for _, s in PACK_ROW)], F32),
        'pack_rowb': ([1, sum(_cols(s) for _, s in PACK_ROWB)], BF16),
        'sel2': ([2 * K, K * K], BF16),
    }
    for name, (shape, dt) in specs.items():
        dr[name] = nc.dram_tensor(name, shape, dt, kind="ExternalInput").ap()
    pred_out = nc.dram_tensor('pred', [n_steps, K, DS], F32, kind="ExternalOutput").ap()
    w_dram = nc.dram_tensor('w_scratch', [n_steps, E, E * P], BF16, kind="Internal").ap()
    dbg_out = None
    if dbg:
        dbg_out = {
            'dbg_h0': nc.dram_tensor('dbg_h0', [P, C, K], F32, kind="ExternalOutput").ap(),
            'dbg_w': nc.dram_tensor('dbg_w', [E, E * P], F32, kind="ExternalOutput").ap(),
            'dbg_xu': nc.dram_tensor('dbg_xu', [P, 2 * C, K], F32, kind="ExternalOutput").ap(),
            'dbg_r1': nc.dram_tensor('dbg_r1', [P, C, K, K], F32, kind="ExternalOutput").ap(),
            'dbg_prod': nc.dram_tensor('dbg_prod', [P, C, K, K], F32, kind="ExternalOutput").ap(),
            'dbg_d0': nc.dram_tensor('dbg_d0', [P, C, K], F32, kind="ExternalOutput").ap(),
        }

    with tile.TileContext(nc) as tc:
        _emit(nc, tc, dr, pred_out, n_steps, dbg_out, w_dram)
    nc.compile()
    return nc


def _emit(nc, tc, dr, pred_out, n_steps, dbg_out=None, w_dram=None):
    from contextlib import ExitStack
    ctx = ExitStack()
    with ctx:
        const = ctx.enter_context(tc.tile_pool(name="const", bufs=1))
        act = ctx.enter_context(tc.tile_pool(name="act", bufs=2))
        big = ctx.enter_context(tc.tile_pool(name="big", bufs=3))
        wpool = ctx.enter_context(tc.tile_pool(name="wpool", bufs=2))
        sm = ctx.enter_context(tc.tile_pool(name="sm", bufs=4))
        dbgp = ctx.enter_context(tc.tile_pool(name="dbgp", bufs=1))
        ps = ctx.enter_context(tc.tile_pool(name="ps", bufs=3, space="PSUM"))
        psb = ctx.enter_context(tc.tile_pool(name="psb", bufs=2, space="PSUM"))
        psd = ctx.enter_context(tc.tile_pool(name="psd", bufs=1, space="PSUM"))

        # ---- load packed constants, build per-name views
        packs = {}
        for pname in ('x0t', 'pack_p8', 'pack_f32a', 'pack_rt16', 'pack_row',
                      'sel2', 'pack_ex16', 'pack_f32b', 'pack_rowb'):
            ap = dr[pname]
            t_ = const.tile(ap.shape, ap.dtype, tag=pname)
            nc.sync.dma_start(out=t_[:], in_=ap[:])
            packs[pname] = t_

        w = {}
        for pname, layout in [('pack_f32a', PACK_F32A), ('pack_f32b', PACK_F32B),
                              ('pack_rt16', PACK_RT16), ('pack_ex16', PACK_EX16),
                              ('pack_p8', PACK_P8), ('pack_row', PACK_ROW),
                              ('pack_rowb', PACK_ROWB)]:
            off = 0
            for name, sh in layout:
                n = _cols(sh)
                view = packs[pname][:sh[0], off:off + n]
                if len(sh) == 3:
                    view = view.rearrange("p (a b) -> p a b", a=sh[1])
                elif len(sh) == 4:
                    view = view.rearrange("p (a b c) -> p a b c", a=sh[1], b=sh[2])
                w[name] = view
                off += n
        sel2 = packs['sel2']

        ones_col = const.tile([P, 1], F32)      # partition-sum matmuls
        nc.vector.memset(ones_col[:], 1.0)
        ones_row = const.tile([1, P], F32)      # partition-broadcast matmuls
        nc.vector.memset(ones_row[:], 1.0)
        ones_tok = const.tile([1, K], F32)      # bias-fold matmuls
        nc.vector.memset(ones_tok[:], 1.0)
        ones_tok_bf = const.tile([1, K], BF16)
        nc.vector.memset(ones_tok_bf[:], 1.0)
        ones8_bf = const.tile([E, 1], BF16)
        nc.vector.memset(ones8_bf[:], 1.0)
        eps_t = const.tile([1, 1], F32)
        nc.vector.memset(eps_t[:], EPS)

        h_f = act.tile([P, C, K], F32, tag="h_f")
        h_b = act.tile([P, C, K], BF16, tag="h_b")

        # ---- helpers ------------------------------------------------------
        def bias_fold(out_ps_mc, brow, mc, width=P, stop=False):
            nc.tensor.matmul(out_ps_mc, brow[:, mc * P:mc * P + width],
                             ones_tok[:], start=False, stop=stop)

        def mlp(x, win1, b1r, win2, b2r, out_sb, cin=C):
            hp = ps.tile([P, C, K], F32, tag="gen")
            for mc in range(C):
                for kc in range(cin):
                    nc.tensor.matmul(hp[:, mc, :],
                                     win1[:, kc, mc * P:(mc + 1) * P], x[:, kc, :],
                                     start=(kc == 0), stop=False)
                bias_fold(hp[:, mc, :], b1r, mc, stop=True)
            hs = sm.tile([P, C, K], F32, tag="mlp_h")
            nc.scalar.activation(hs[:], hp[:], ACTF.Relu)
            op = ps.tile([P, C, K], F32, tag="gen")
            for mc in range(C):
                for kc in range(C):
                    nc.tensor.matmul(op[:, mc, :],
                                     win2[:, kc, mc * P:(mc + 1) * P], hs[:, kc, :],
                                     start=(kc == 0), stop=False)
                bias_fold(op[:, mc, :], b2r, mc, stop=True)
            nc.scalar.activation(out_sb, op[:], ACTF.Copy)
            return out_sb

        def layer_norm(x, g, bb, cin, out):
            sq = sm.tile([P, cin, K], F32, tag="ln_sq")
            nc.vector.tensor_mul(sq[:], x[:], x[:])
            mp = ps.tile([1, K], F32, tag="gen")
            qp = ps.tile([1, K], F32, tag="gen")
            for kc in range(cin):
                nc.tensor.matmul(mp[:], ones_col[:], x[:, kc, :],
                                 start=(kc == 0), stop=(kc == cin - 1))
            for kc in range(cin):
                nc.tensor.matmul(qp[:], ones_col[:], sq[:, kc, :],
                                 start=(kc == 0), stop=(kc == cin - 1))
            st = sm.tile([1, 2, K], F32, tag="ln_st")
            dinv = 1.0 / (cin * P)
            nc.vector.tensor_scalar(out=st[:, 0, :], in0=mp[:], scalar1=dinv,
                                    scalar2=None, op0=ALU.mult)
            m2 = sm.tile([1, K], F32, tag="ln_m2")
            nc.vector.tensor_scalar(out=m2[:], in0=qp[:], scalar1=dinv,
                                    scalar2=None, op0=ALU.mult)
            var = sm.tile([1, K], F32, tag="ln_var")
            nc.vector.tensor_mul(var[:], st[:, 0, :], st[:, 0, :])
            nc.vector.tensor_sub(var[:], m2[:], var[:])
            std = sm.tile([1, K], F32, tag="ln_std")
            nc.scalar.activation(std[:], var[:], ACTF.Sqrt, bias=eps_t[:])
            nc.vector.reciprocal(st[:, 1, :], std[:])
            bp = ps.tile([P, 2, K], F32, tag="gen")
            nc.tensor.matmul(bp[:, 0, :], ones_row[:], st[:, 0, :], start=True, stop=True)
            nc.tensor.matmul(bp[:, 1, :], ones_row[:], st[:, 1, :], start=True, stop=True)
            t1 = sm.tile([P, cin, K], F32, tag="ln_t1")
            mb = bp[:, 0:1, :].broadcast_to([P, cin, K])
            rb = bp[:, 1:2, :].broadcast_to([P, cin, K])
            nc.vector.tensor_sub(t1[:], x[:], mb)
            nc.vector.tensor_mul(t1[:], t1[:], rb)
            for c_ in range(cin):
                nc.vector.tensor_scalar(out=out[:, c_, :], in0=t1[:, c_, :],
                                        scalar1=g[:, c_:c_ + 1],
                                        scalar2=bb[:, c_:c_ + 1],
                                        op0=ALU.mult, op1=ALU.add)
            return out

        def encode(x_sb):
            hp = ps.tile([P, C, K], F32, tag="gen")
            for mc in range(C):
                nc.tensor.matmul(hp[:, mc, :], w['enc_w1'][:, mc * P:(mc + 1) * P],
                                 x_sb[:], start=True, stop=False)
                bias_fold(hp[:, mc, :], w['enc_b1r'], mc, stop=True)
            hs = sm.tile([P, C, K], F32, tag="enc_h")
            nc.scalar.activation(hs[:], hp[:], ACTF.Relu)
            op = ps.tile([P, C, K], F32, tag="gen")
            for mc in range(C):
                for kc in range(C):
                    nc.tensor.matmul(op[:, mc, :],
                                     w['enc_w2'][:, kc, mc * P:(mc + 1) * P],
                                     hs[:, kc, :], start=(kc == 0), stop=False)
                bias_fold(op[:, mc, :], w['enc_b2r'], mc, stop=True)
            nc.scalar.activation(h_f[:], op[:], ACTF.Copy)
            nc.scalar.activation(h_b[:], op[:], ACTF.Copy)

        # ---- initial encode
        x0 = packs['x0t']
        encode(x0)
        if dbg_out is not None:
            nc.sync.dma_start(out=dbg_out['dbg_h0'][:], in_=h_f[:])

        # ---- rollout steps
        for t in range(n_steps):
            # ---------- router ----------
            mulij = big.tile([P, C, K, K], BF16, tag="mulij")
            for kc in range(C):
                nc.vector.tensor_mul(
                    mulij[:, kc],
                    h_b[:, kc, None, :].broadcast_to([P, K, K]),
                    h_b[:, kc, :, None].broadcast_to([P, K, K]))

            arp = ps.tile([P, C, K], F32, tag="gen")
            brp = ps.tile([P, C, K], F32, tag="gen")
            for mc in range(C):
                for kc in range(C):
                    nc.tensor.matmul(arp[:, mc, :],
                                     w['rt_wa'][:, kc, mc * P:(mc + 1) * P],
                                     h_f[:, kc, :], start=(kc == 0), stop=False)
                bias_fold(arp[:, mc, :], w['rt_b1r'], mc, stop=True)
                for kc in range(C):
                    nc.tensor.matmul(brp[:, mc, :],
                                     w['rt_wb'][:, kc, mc * P:(mc + 1) * P],
                                     h_f[:, kc, :], start=(kc == 0),
                                     stop=(kc == C - 1))
            ar = sm.tile([P, C, K], BF16, tag="ar")
            br = sm.tile([P, C, K], BF16, tag="br")
            nc.scalar.activation(ar[:], arp[:], ACTF.Copy)
            nc.scalar.activation(br[:], brp[:], ACTF.Copy)

            r1 = big.tile([P, C, K, K], BF16, tag="r1")
            for mc in range(C):
                cp = psb.tile([P, K, K], F32, tag="pbig")
                for kc in range(C):
                    for nh in range(2):
                        nc.tensor.matmul(
                            cp[:, nh * 16:(nh + 1) * 16, :],
                            w['rt_wd'][:, kc, mc * P:(mc + 1) * P],
                            mulij[:, kc, nh * 16:(nh + 1) * 16, :],
                            start=(kc == 0), stop=(kc == C - 1))
                pre = big.tile([P, K, K], BF16, tag="rpre")
                nc.gpsimd.tensor_tensor(
                    pre[:], ar[:, mc, None, :].broadcast_to([P, K, K]),
                    br[:, mc, :, None].broadcast_to([P, K, K]), op=ALU.add)
                nc.vector.tensor_add(pre[:], pre[:], cp[:])
                nc.scalar.activation(r1[:, mc, :, :], pre[:], ACTF.Relu)

            # feature-major logits [8, 1024]; softmax over the 8 partitions.
            # |logits| << 1 (0.02-scale weights), so skip the max-subtract;
            # exp(logits + b2) fuses into one ACT op via per-partition bias.
            lg = psb.tile([E, 2, 512], F32, tag="pbig")
            for nh in range(2):
                for kc in range(C):
                    nc.tensor.matmul(
                        lg[:, nh, :], w['rt_w2'][:, kc, :],
                        r1[:, kc, :, :].rearrange("p a b -> p (a b)")
                        [:, nh * 512:(nh + 1) * 512],
                        start=(kc == 0), stop=(kc == C - 1))
            ez = wpool.tile([E, E * P], BF16, tag="ez")
            nc.scalar.activation(ez[:].rearrange("e (a b) -> e a b", a=2),
                                 lg[:], ACTF.Exp, bias=w['rt_b2c'][:])
            rc = wpool.tile([1, E * P], F32, tag="rc")
            for nh in range(2):
                smp = ps.tile([1, 512], F32, tag="gen")
                nc.tensor.matmul(smp[:], ones8_bf[:],
                                 ez[:].rearrange("e (a b) -> e a b", a=2)[:, nh, :],
                                 start=True, stop=True)
                nc.scalar.activation(rc[:, nh * 512:(nh + 1) * 512], smp[:],
                                     ACTF.Copy)
            nc.vector.reciprocal(rc[:], rc[:])
            rcb = wpool.tile([1, E * P], BF16, tag="rcb")
            nc.scalar.activation(rcb[:], rc[:], ACTF.Copy)
            rc_bc = wpool.tile([E, E * P], BF16, tag="rc_bc")
            nc.gpsimd.partition_broadcast(rc_bc[:], rcb[:])
            w_all = act.tile([E, E * P], BF16, tag="w_all")
            nc.vector.tensor_mul(w_all[:], ez[:], rc_bc[:])
            nc.sync.dma_start(out=w_dram[t], in_=w_all[:])
            s_sb = sm.tile([E, K], F32, tag="s_sb")
            nc.vector.tensor_reduce(
                s_sb[:], w_all[:].rearrange("e (a b) -> e a b", a=K),
                axis=AX.X, op=ALU.add)
            if dbg_out is not None and t == 0:
                dw = dbgp.tile([E, E * P], F32, tag="dbg_dw")
                nc.scalar.activation(dw[:], w_all[:], ACTF.Copy)
                nc.sync.dma_start(out=dbg_out['dbg_w'][:], in_=dw[:])
                dr1 = dbgp.tile([P, C, K, K], F32, tag="dbg_r1t")
                nc.scalar.activation(dr1[:], r1[:], ACTF.Copy)
                nc.sync.dma_start(out=dbg_out['dbg_r1'][:], in_=dr1[:])

            # all-expert broadcast of w rows across 128 partitions (one DMA)
            wbca = wpool.tile([P, E, E * P], BF16, tag="wbca")
            wsrc = w_dram[t].rearrange("e q -> (e q)")
            wsrc = bass.AP(tensor=wsrc.tensor, offset=wsrc.offset,
                           ap=[[0, P]] + list(wsrc.ap))
            nc.sync.dma_start(out=wbca[:], in_=wsrc)

            # ---------- delta_self ----------
            xfs = sm.tile([P, C, K], F32, tag="xfs")
            layer_norm(h_f, w['fs_g'], w['fs_bb'], C, xfs)
            xu = act.tile([P, 2 * C, K], F32, tag="xu")
            mlp(xfs, w['fs_w1'], w['fs_b1r'], w['fs_w2'], w['fs_b2r'],
                xu[:, 0:C, :])

            # ---------- experts ----------
            dpsa = psd.tile([P, C, K], F32, tag="delta")
            dps = [dpsa[:, 0, :], dpsa[:, 1, :]]
            nc.vector.memset(dpsa[:], 0.0)
            prev_g2 = [None]
            for e in range(E):
                # token-major A^T/B^T: lhsT = h chunk, rhs = expert weights
                abp = ps.tile([K, 2 * D], F32, tag="gen")
                for kc in range(C):
                    nc.tensor.matmul(abp[:], h_b[:, kc, :],
                                     w['ex_wsij'][:, kc, e, :],
                                     start=(kc == 0), stop=False)
                # b1 folded into the B rows: each pair column picks one B row
                nc.tensor.matmul(abp[:], ones_tok_bf[:], w['ex_b1z'][0:1, e, :],
                                 start=False, stop=True)
                ab = sm.tile([2 * K, C, P], BF16, tag="ab")
                nc.scalar.activation(ab[0:K], abp[:, 0:D].rearrange(
                    "k (c p) -> k c p", c=C), ACTF.Copy)
                nc.scalar.activation(ab[K:2 * K], abp[:, D:2 * D].rearrange(
                    "k (c p) -> k c p", c=C), ACTF.Copy)

                prod = big.tile([P, C, K, K], BF16, tag="exprod")
                wv_ = wbca[:, e, :].rearrange("p (b c) -> p b c", b=K)
                for mc in range(C):
                    # pre[d,(j,i)] = A[d,i] + B[d,j] + b1[d] via selector matmul
                    pp_ = psb.tile([P, K, K], F32, tag="pbig")
                    for nh in range(2):
                        nc.tensor.matmul(pp_[:, nh * 16:(nh + 1) * 16, :],
                                         ab[:, mc, :],
                                         sel2[:, nh * 512:(nh + 1) * 512],
                                         start=True, stop=True)
                    if e % 3 == 0:
                        # fused relu+w-mul on DVE: prod = max(pre,0)*w
                        nc.vector.scalar_tensor_tensor(
                            out=prod[:, mc], in0=pp_[:], scalar=0.0, in1=wv_,
                            op0=ALU.max, op1=ALU.mult)
                    else:
                        h1m = big.tile([P, K, K], BF16, tag="exh1")
                        nc.scalar.activation(h1m[:], pp_[:], ACTF.Relu)
                        nc.vector.tensor_mul(prod[:, mc], h1m[:], wv_)
                pr16 = big.tile([P, C, K, 16], BF16, tag="expr16")
                nc.vector.tensor_add(pr16[:], prod[:, :, :, 0:16],
                                     prod[:, :, :, 16:32])
                pr8 = big.tile([P, C, K, 8], BF16, tag="expr8")
                nc.vector.tensor_add(pr8[:], pr16[:, :, :, 0:8],
                                     pr16[:, :, :, 8:16])
                pr4 = big.tile([P, C, K, NFOLD], BF16, tag="expr4")
                nc.vector.tensor_add(pr4[:], pr8[:, :, :, 0:NFOLD],
                                     pr8[:, :, :, NFOLD:2 * NFOLD])

                for kc in range(C):
                    for mc in range(C):
                        for i in range(NFOLD):
                            mm = nc.tensor.matmul(
                                dps[mc],
                                w['ex_w2'][:, kc, e, mc * P:(mc + 1) * P],
                                pr4[:, kc, :, i],
                                start=False, stop=False,
                                skip_group_check=True)
                            if prev_g2[0] is not None:
                                add_dep_helper(mm.ins, prev_g2[0], sync=False,
                                               reason="g2-weight-run order")
                            prev_g2[0] = mm.ins
                if dbg_out is not None and t == 0 and e == 0:
                    dpr = dbgp.tile([P, C, K, K], F32, tag="dbg_prod_t")
                    nc.scalar.activation(dpr[:], prod[:], ACTF.Copy)
                    nc.sync.dma_start(out=dbg_out['dbg_prod'][:], in_=dpr[:])
                    dd0 = dbgp.tile([P, C, K], F32, tag="dbg_d0_t")
                    nc.scalar.activation(dd0[:, 0, :], dps[0], ACTF.Copy)
                    nc.scalar.activation(dd0[:, 1, :], dps[1], ACTF.Copy)
                    nc.sync.dma_start(out=dbg_out['dbg_d0'][:], in_=dd0[:])
            for mc in range(C):
                nc.tensor.matmul(dps[mc], w['ex_b2t'][:, mc * P:(mc + 1) * P],
                                 s_sb[:], start=False, stop=True,
                                 skip_group_check=True)

            # ---------- update ----------
            nc.scalar.activation(xu[:, C, :], dps[0], ACTF.Copy)
            nc.scalar.activation(xu[:, C + 1, :], dps[1], ACTF.Copy)
            if dbg_out is not None and t == 0:
                nc.sync.dma_start(out=dbg_out['dbg_xu'][:], in_=xu[:])
            xln = sm.tile([P, 2 * C, K], F32, tag="xln")
            layer_norm(xu, w['up_g'], w['up_bb'], 2 * C, xln)
            hnew = act.tile([P, C, K], F32, tag="hnew")
            mlp(xln, w['up_w1'], w['up_b1r'], w['up_w2'], w['up_b2r'], hnew,
                cin=2 * C)

            # ---------- decode ----------
            hd = ps.tile([P, C, K], F32, tag="gen")
            for mc in range(C):
                for kc in range(C):
                    nc.tensor.matmul(hd[:, mc, :],
                                     w['dec_w1'][:, kc, mc * P:(mc + 1) * P],
                                     hnew[:, kc, :], start=(kc == 0), stop=False)
                bias_fold(hd[:, mc, :], w['dec_b1r'], mc, stop=True)
            hds = sm.tile([P, C, K], F32, tag="dec_h")
            nc.scalar.activation(hds[:], hd[:], ACTF.Relu)
            pp = ps.tile([DS, K], F32, tag="gen")
            for kc in range(C):
                nc.tensor.matmul(pp[:], w['dec_w2'][:, kc, :], hds[:, kc, :],
                                 start=(kc == 0), stop=False)
            nc.tensor.matmul(pp[:], w['dec_b2r'][:], ones_tok[:],
                             start=False, stop=True)
            pred_sb = act.tile([DS, K], F32, tag="pred_sb")
            nc.scalar.activation(pred_sb[:], pp[:], ACTF.Copy)
            nc.sync.dma_start(out=pred_out[t].rearrange("k s -> s k"),
                              in_=pred_sb[:])

            if t < n_steps - 1:
                encode(pred_sb)


# --------------------------------------------------------------------------
# host wrapper
# --------------------------------------------------------------------------

_PROG_CACHE = {}


def _get_program(n_steps):
    if n_steps not in _PROG_CACHE:
        _PROG_CACHE[n_steps] = build_program(n_steps)
    return _PROG_CACHE[n_steps]


def kernel(gt_states, params, rollout_steps):
    from concourse.bass_utils import run_bass_kernel_spmd

    gt = np.asarray(gt_states, np.float32)
    B, T, K_, Ds_ = gt.shape
    n_steps = min(T - 1, int(rollout_steps))
    nc = _get_program(n_steps)

    shared = prep_params(params)
    in_maps = []
    for b in range(B):
        m = dict(shared)
        m['x0t'] = np.ascontiguousarray(gt[b, 0].T)   # [8, 32]
        in_maps.append(m)

    res = run_bass_kernel_spmd(nc, in_maps, core_ids=list(range(B)))
    pred = np.stack([res.results[b]['pred'] for b in range(B)], 0)
    target = gt[:, 1:n_steps + 1]
    return pred.astype(np.float32), target


# revision 48
# speedup vs baseline: 1.2281x; 1.0132x over previous
"""Trainium2 Bass kernel for nn_MoEPairwise (MoE pairwise routing rollout).

Strategy
--------
Data-parallel over batch B=8 == 8 NeuronCores; zero collectives. Each core
runs the full 2-step rollout for one batch element (K=32 agents, D=256,
E=8 experts).

Math factorization (validated vs reference in numpy + CoreSim):
  - pair_in = [si, sj] so expert GEMM1 factors: h1_e = relu(A_e[.,i] + B_e[.,j] + b1)
    with A_e = W1si_e^T h, B_e = W1sj_e^T h  (GEMMs over 32 tokens, not 1024 pairs).
    A^T/B^T are computed token-major (lhsT = h) and the (i,j) broadcast-add runs
    on the PE as [A^T;B^T]^T @ SEL2 with a constant 0/1 selector, writing the
    1024-wide pre-activation straight into PSUM. b1 folds into the relu-evict
    bias (per-partition), w-multiply is a dense bf16 DVE op against a
    DMA-broadcast weight plane.
  - softmax weights w >= 0, so the weighted reduction folds over the pair grid:
    delta_inter = sum_e W2_e^T (sum_i w_e(j,i) * h1_e[:, (j,i)]) + b2^T s.
    i is folded 32->8 by two dense DVE adds, then contracted on the PE with
    8 accumulating matmuls per (e,kc,mc) into per-mc PSUM tiles (PSUM
    accumulation groups must not share banks).
  - router GEMM1 factors the si/sj/si-sj rows the same way; only the si*sj
    term needs a real GEMM over the 1024-pair grid.

Layouts: feature-major activations [128 partitions = d % 128, chunk, token].
Pair-grid columns ordered (j outer, i inner). bf16 for the wide expert/router
path (PE fp32 is 4x slower; DVE tensor ops 2x faster in bf16), fp32 for the
narrow backbone (LN, small MLPs) and all PSUM accumulation.
"""

import numpy as np
import ml_dtypes

import concourse.bass as bass
import concourse.mybir as mybir
import concourse.tile as tile
from concourse import bacc
from concourse.masks import make_identity
from concourse.tile_rust import add_dep_helper

F32 = mybir.dt.float32
BF16 = mybir.dt.bfloat16
AX = mybir.AxisListType
ALU = mybir.AluOpType
ACTF = mybir.ActivationFunctionType

K = 32          # agents (tokens per batch)
D = 256         # hidden
E = 8           # experts
DS = 8          # state dim
P = 128         # partitions
C = D // P      # feature chunks (2)
EPS = 1e-5
NFOLD = 2       # folded i-groups entering the G2 contraction

BF = np.dtype(ml_dtypes.bfloat16)

# packed-constant layouts: name -> (view_shape, ncols) appended in order.
# pack tiles are [npart, total_cols]; views slice columns and rearrange.
PACK_F32A = [         # [128, .] fp32, needed early (enc/router/fs)
    ('enc_w2', (P, C, D)), ('rt_wa', (P, C, D)), ('rt_wb', (P, C, D)),
    ('fs_w1', (P, C, D)), ('fs_w2', (P, C, D)),
    ('fs_g', (P, C)), ('fs_bb', (P, C)), ('sel', (P, 4)),
]
PACK_F32B = [         # [128, .] fp32, needed late (experts/update/dec)
    ('up_w1', (P, 2 * C, D)), ('up_w2', (P, C, D)),
    ('dec_w1', (P, C, D)), ('dec_w2', (P, C, DS)),
    ('up_g', (P, 2 * C)), ('up_bb', (P, 2 * C)),
    ('ex_b1', (P, C, E)),
]
PACK_RT16 = [         # [128, .] bf16, router
    ('rt_wd', (P, C, D)), ('rt_w2', (P, C, E)),
]
PACK_EX16 = [         # [128, .] bf16, experts
    ('ex_wsij', (P, C, E, 2 * D)), ('ex_w2', (P, C, E, D)),
]
PACK_P8 = [           # [8, .] fp32
    ('enc_w1', (DS, D)), ('ex_b2t', (E, D)), ('rt_b2c', (E, 1)),
]
PACK_ROWB = [         # [1, .] bf16
    ('ex_b1z', (1, E, 2 * D)),
]
PACK_ROW = [          # [1, .] fp32
    ('enc_b1r', (1, D)), ('enc_b2r', (1, D)),
    ('fs_b1r', (1, D)), ('fs_b2r', (1, D)),
    ('rt_b1r', (1, D)), ('rt_b2r', (1, E)),
    ('up_b1r', (1, D)), ('up_b2r', (1, D)),
    ('dec_b1r', (1, D)), ('dec_b2r', (1, DS)),
]


def _cols(shape):
    n = 1
    for s in shape[1:]:
        n *= s
    return n


# --------------------------------------------------------------------------
# host-side parameter preprocessing
# --------------------------------------------------------------------------

def prep_params(params):
    p = {k: np.asarray(v, np.float32) for k, v in params.items()}
    t = {}

    def chunked(x, cin=None):  # [d_in, d_out] -> [128, cin, d_out]
        d_in = x.shape[0]
        cin = d_in // P
        return x.reshape(cin, P, -1).transpose(1, 0, 2)

    t['enc_w1'] = p['enc_W1']
    t['enc_w2'] = chunked(p['enc_W2'])
    t['fs_w1'] = chunked(p['fs_W1'])
    t['fs_w2'] = chunked(p['fs_W2'])
    Wa = p['rt_W1'][0:D] + p['rt_W1'][2 * D:3 * D]
    Wb = p['rt_W1'][D:2 * D] - p['rt_W1'][2 * D:3 * D]
    t['rt_wa'] = chunked(Wa)
    t['rt_wb'] = chunked(Wb)
    t['rt_wd'] = chunked(p['rt_W1'][3 * D:4 * D])
    t['rt_w2'] = chunked(p['rt_W2'])
    wsi = p['ex_W1'][:, 0:D, :].reshape(E, C, P, D).transpose(2, 1, 0, 3)
    wsj = p['ex_W1'][:, D:2 * D, :].reshape(E, C, P, D).transpose(2, 1, 0, 3)
    t['ex_wsij'] = np.concatenate([wsi, wsj], axis=-1)   # [128, C, E, 512]
    t['ex_w2'] = p['ex_W2'].reshape(E, C, P, D).transpose(2, 1, 0, 3)
    t['ex_b1'] = p['ex_b1'].T.reshape(C, P, E).transpose(1, 0, 2)   # [128, C, E]
    t['ex_b1z'] = np.concatenate([np.zeros_like(p['ex_b1']), p['ex_b1']],
                                 axis=1)[None]                      # [1, E, 512]
    t['ex_b2t'] = p['ex_b2']
    t['rt_b2c'] = p['rt_b2'][:, None]
    t['up_w1'] = chunked(p['up_W1'])
    t['up_w2'] = chunked(p['up_W2'])
    t['dec_w1'] = chunked(p['dec_W1'])
    t['dec_w2'] = chunked(p['dec_W2'])
    t['fs_g'] = p['fs_g'].reshape(C, P).T
    t['fs_bb'] = p['fs_b'].reshape(C, P).T
    t['up_g'] = p['up_g'].reshape(2 * C, P).T
    t['up_bb'] = p['up_b'].reshape(2 * C, P).T
    for n_, b_ in [('enc_b1r', 'enc_b1'), ('enc_b2r', 'enc_b2'),
                   ('fs_b1r', 'fs_b1'), ('fs_b2r', 'fs_b2'),
                   ('rt_b1r', 'rt_b1'), ('rt_b2r', 'rt_b2'),
                   ('up_b1r', 'up_b1'), ('up_b2r', 'up_b2'),
                   ('dec_b1r', 'dec_b1'), ('dec_b2r', 'dec_b2')]:
        t[n_] = p[b_][None, :]
    sel = np.zeros((P, 4), np.float32)
    for jl in range(4):
        sel[jl * K:(jl + 1) * K, jl] = 1.0
    t['sel'] = sel

    def pack(layout, npart, dtype):
        cols = sum(_cols(sh) for _, sh in layout)
        buf = np.zeros((npart, cols), np.float32)
        off = 0
        for name, sh in layout:
            n = _cols(sh)
            arr = t[name]
            assert tuple(arr.shape) == tuple(sh), (name, arr.shape, sh)
            buf[:sh[0], off:off + n] = arr.reshape(sh[0], n)
            off += n
        return np.ascontiguousarray(buf.astype(dtype))

    ins = {
        'pack_f32a': pack(PACK_F32A, P, np.float32),
        'pack_f32b': pack(PACK_F32B, P, np.float32),
        'pack_rt16': pack(PACK_RT16, P, BF),
        'pack_ex16': pack(PACK_EX16, P, BF),
        'pack_p8': pack(PACK_P8, DS, np.float32),
        'pack_row': pack(PACK_ROW, 1, np.float32),
        'pack_rowb': pack(PACK_ROWB, 1, BF),
    }
    # SEL2 [64, 1024]: row k<32 selects i==k; row k>=32 selects j==k-32
    sel2 = np.zeros((2 * K, K * K), np.float32)
    for i in range(K):
        sel2[i, i::K] = 1.0          # columns (j, i) with that i
        sel2[K + i, i * K:(i + 1) * K] = 1.0   # columns with j == i
    ins['sel2'] = np.ascontiguousarray(sel2.astype(BF))
    return ins


# --------------------------------------------------------------------------
# device program
# --------------------------------------------------------------------------

def build_program(n_steps=2, dbg=False):
    nc = bacc.Bacc("TRN2", target_bir_lowering=False, debug=False)

    dr = {}
    specs = {
        'x0t': ([DS, K], F32),
        'pack_f32a': ([P, sum(_cols(s) for _, s in PACK_F32A)], F32),
        'pack_f32b': ([P, sum(_cols(s) for _, s in PACK_F32B)], F32),
        'pack_rt16': ([P, sum(_cols(s) for _, s in PACK_RT16)], BF16),
        'pack_ex16': ([P, sum(_cols(s) for _, s in PACK_EX16)], BF16),
        'pack_p8': ([DS, sum(_cols(s) for _, s in PACK_P8)], F32),
        'pack_row': ([1, sum(_cols(s) for _, s in PACK_ROW)], F32),
        'pack_rowb': ([1, sum(_cols(s) for _, s in PACK_ROWB)], BF16),
        'sel2': ([2 * K, K * K], BF16),
    }
    for name, (shape, dt) in specs.items():
        dr[name] = nc.dram_tensor(name, shape, dt, kind="ExternalInput").ap()
    pred_out = nc.dram_tensor('pred', [n_steps, K, DS], F32, kind="ExternalOutput").ap()
    w_dram = nc.dram_tensor('w_scratch', [n_steps, E, E * P], BF16, kind="Internal").ap()
    dbg_out = None
    if dbg:
        dbg_out = {
            'dbg_h0': nc.dram_tensor('dbg_h0', [P, C, K], F32, kind="ExternalOutput").ap(),
            'dbg_w': nc.dram_tensor('dbg_w', [E, E * P], F32, kind="ExternalOutput").ap(),
            'dbg_xu': nc.dram_tensor('dbg_xu', [P, 2 * C, K], F32, kind="ExternalOutput").ap(),
            'dbg_r1': nc.dram_tensor('dbg_r1', [P, C, K, K], F32, kind="ExternalOutput").ap(),
            'dbg_prod': nc.dram_tensor('dbg_prod', [P, C, K, K], F32, kind="ExternalOutput").ap(),
            'dbg_d0': nc.dram_tensor('dbg_d0', [P, C, K], F32, kind="ExternalOutput").ap(),
        }

    with tile.TileContext(nc) as tc:
        _emit(nc, tc, dr, pred_out, n_steps, dbg_out, w_dram)
    nc.compile()
    return nc


def _emit(nc, tc, dr, pred_out, n_steps, dbg_out=None, w_dram=None):
    from contextlib import ExitStack
    ctx = ExitStack()
    with ctx:
        const = ctx.enter_context(tc.tile_pool(name="const", bufs=1))
        act = ctx.enter_context(tc.tile_pool(name="act", bufs=2))
        big = ctx.enter_context(tc.tile_pool(name="big", bufs=3))
        wpool = ctx.enter_context(tc.tile_pool(name="wpool", bufs=2))
        sm = ctx.enter_context(tc.tile_pool(name="sm", bufs=4))
        dbgp = ctx.enter_context(tc.tile_pool(name="dbgp", bufs=1))
        ps = ctx.enter_context(tc.tile_pool(name="ps", bufs=3, space="PSUM"))
        psb = ctx.enter_context(tc.tile_pool(name="psb", bufs=2, space="PSUM"))
        psd = ctx.enter_context(tc.tile_pool(name="psd", bufs=1, space="PSUM"))

        # ---- load packed constants, build per-name views
        packs = {}
        for pname in ('x0t', 'pack_p8', 'pack_f32a', 'pack_rt16', 'pack_row',
                      'sel2', 'pack_ex16', 'pack_f32b', 'pack_rowb'):
            ap = dr[pname]
            t_ = const.tile(ap.shape, ap.dtype, tag=pname)
            nc.sync.dma_start(out=t_[:], in_=ap[:])
            packs[pname] = t_

        w = {}
        for pname, layout in [('pack_f32a', PACK_F32A), ('pack_f32b', PACK_F32B),
                              ('pack_rt16', PACK_RT16), ('pack_ex16', PACK_EX16),
                              ('pack_p8', PACK_P8), ('pack_row', PACK_ROW),
                              ('pack_rowb', PACK_ROWB)]:
            off = 0
            for name, sh in layout:
                n = _cols(sh)
                view = packs[pname][:sh[0], off:off + n]
                if len(sh) == 3:
                    view = view.rearrange("p (a b) -> p a b", a=sh[1])
                elif len(sh) == 4:
                    view = view.rearrange("p (a b c) -> p a b c", a=sh[1], b=sh[2])
                w[name] = view
                off += n
        sel2 = packs['sel2']

        ones_col = const.tile([P, 1], F32)      # partition-sum matmuls
        nc.vector.memset(ones_col[:], 1.0)
        ones_row = const.tile([1, P], F32)      # partition-broadcast matmuls
        nc.vector.memset(ones_row[:], 1.0)
        ones_tok = const.tile([1, K], F32)      # bias-fold matmuls
        nc.vector.memset(ones_tok[:], 1.0)
        ones_tok_bf = const.tile([1, K], BF16)
        nc.vector.memset(ones_tok_bf[:], 1.0)
        ones8_bf = const.tile([E, 1], BF16)
        nc.vector.memset(ones8_bf[:], 1.0)
        eps_t = const.tile([1, 1], F32)
        nc.vector.memset(eps_t[:], EPS)

        h_f = act.tile([P, C, K], F32, tag="h_f")
        h_b = act.tile([P, C, K], BF16, tag="h_b")

        # ---- helpers ------------------------------------------------------
        def bias_fold(out_ps_mc, brow, mc, width=P, stop=False):
            nc.tensor.matmul(out_ps_mc, brow[:, mc * P:mc * P + width],
                             ones_tok[:], start=False, stop=stop)

        def mlp(x, win1, b1r, win2, b2r, out_sb, cin=C):
            hp = ps.tile([P, C, K], F32, tag="gen")
            for mc in range(C):
                for kc in range(cin):
                    nc.tensor.matmul(hp[:, mc, :],
                                     win1[:, kc, mc * P:(mc + 1) * P], x[:, kc, :],
                                     start=(kc == 0), stop=False)
                bias_fold(hp[:, mc, :], b1r, mc, stop=True)
            hs = sm.tile([P, C, K], F32, tag="mlp_h")
            nc.scalar.activation(hs[:], hp[:], ACTF.Relu)
            op = ps.tile([P, C, K], F32, tag="gen")
            for mc in range(C):
                for kc in range(C):
                    nc.tensor.matmul(op[:, mc, :],
                                     win2[:, kc, mc * P:(mc + 1) * P], hs[:, kc, :],
                                     start=(kc == 0), stop=False)
                bias_fold(op[:, mc, :], b2r, mc, stop=True)
            nc.scalar.activation(out_sb, op[:], ACTF.Copy)
            return out_sb

        def layer_norm(x, g, bb, cin, out):
            sq = sm.tile([P, cin, K], F32, tag="ln_sq")
            nc.vector.tensor_mul(sq[:], x[:], x[:])
            mp = ps.tile([1, K], F32, tag="gen")
            qp = ps.tile([1, K], F32, tag="gen")
            for kc in range(cin):
                nc.tensor.matmul(mp[:], ones_col[:], x[:, kc, :],
                                 start=(kc == 0), stop=(kc == cin - 1))
            for kc in range(cin):
                nc.tensor.matmul(qp[:], ones_col[:], sq[:, kc, :],
                                 start=(kc == 0), stop=(kc == cin - 1))
            st = sm.tile([1, 2, K], F32, tag="ln_st")
            dinv = 1.0 / (cin * P)
            nc.vector.tensor_scalar(out=st[:, 0, :], in0=mp[:], scalar1=dinv,
                                    scalar2=None, op0=ALU.mult)
            m2 = sm.tile([1, K], F32, tag="ln_m2")
            nc.vector.tensor_scalar(out=m2[:], in0=qp[:], scalar1=dinv,
                                    scalar2=None, op0=ALU.mult)
            var = sm.tile([1, K], F32, tag="ln_var")
            nc.vector.tensor_mul(var[:], st[:, 0, :], st[:, 0, :])
            nc.vector.tensor_sub(var[:], m2[:], var[:])
            std = sm.tile([1, K], F32, tag="ln_std")
            nc.scalar.activation(std[:], var[:], ACTF.Sqrt, bias=eps_t[:])
            nc.vector.reciprocal(st[:, 1, :], std[:])
            bp = ps.tile([P, 2, K], F32, tag="gen")
            nc.tensor.matmul(bp[:, 0, :], ones_row[:], st[:, 0, :], start=True, stop=True)
            nc.tensor.matmul(bp[:, 1, :], ones_row[:], st[:, 1, :], start=True, stop=True)
            t1 = sm.tile([P, cin, K], F32, tag="ln_t1")
            mb = bp[:, 0:1, :].broadcast_to([P, cin, K])
            rb = bp[:, 1:2, :].broadcast_to([P, cin, K])
            nc.vector.tensor_sub(t1[:], x[:], mb)
            nc.vector.tensor_mul(t1[:], t1[:], rb)
            for c_ in range(cin):
                nc.vector.tensor_scalar(out=out[:, c_, :], in0=t1[:, c_, :],
                                        scalar1=g[:, c_:c_ + 1],
                                        scalar2=bb[:, c_:c_ + 1],
                                        op0=ALU.mult, op1=ALU.add)
            return out

        def encode(x_sb):
            hp = ps.tile([P, C, K], F32, tag="gen")
            for mc in range(C):
                nc.tensor.matmul(hp[:, mc, :], w['enc_w1'][:, mc * P:(mc + 1) * P],
                                 x_sb[:], start=True, stop=False)
                bias_fold(hp[:, mc, :], w['enc_b1r'], mc, stop=True)
            hs = sm.tile([P, C, K], F32, tag="enc_h")
            nc.scalar.activation(hs[:], hp[:], ACTF.Relu)
            op = ps.tile([P, C, K], F32, tag="gen")
            for mc in range(C):
                for kc in range(C):
                    nc.tensor.matmul(op[:, mc, :],
                                     w['enc_w2'][:, kc, mc * P:(mc + 1) * P],
                                     hs[:, kc, :], start=(kc == 0), stop=False)
                bias_fold(op[:, mc, :], w['enc_b2r'], mc, stop=True)
            nc.scalar.activation(h_f[:], op[:], ACTF.Copy)
            nc.scalar.activation(h_b[:], op[:], ACTF.Copy)

        # ---- initial encode
        x0 = packs['x0t']
        encode(x0)
        if dbg_out is not None:
            nc.sync.dma_start(out=dbg_out['dbg_h0'][:], in_=h_f[:])

        # ---- rollout steps
        for t in range(n_steps):
            # ---------- router ----------
            mulij = big.tile([P, C, K, K], BF16, tag="mulij")
            for kc in range(C):
                nc.vector.tensor_mul(
                    mulij[:, kc],
                    h_b[:, kc, None, :].broadcast_to([P, K, K]),
                    h_b[:, kc, :, None].broadcast_to([P, K, K]))

            arp = ps.tile([P, C, K], F32, tag="gen")
            brp = ps.tile([P, C, K], F32, tag="gen")
            for mc in range(C):
                for kc in range(C):
                    nc.tensor.matmul(arp[:, mc, :],
                                     w['rt_wa'][:, kc, mc * P:(mc + 1) * P],
                                     h_f[:, kc, :], start=(kc == 0), stop=False)
                bias_fold(arp[:, mc, :], w['rt_b1r'], mc, stop=True)
                for kc in range(C):
                    nc.tensor.matmul(brp[:, mc, :],
                                     w['rt_wb'][:, kc, mc * P:(mc + 1) * P],
                                     h_f[:, kc, :], start=(kc == 0),
                                     stop=(kc == C - 1))
            ar = sm.tile([P, C, K], BF16, tag="ar")
            br = sm.tile([P, C, K], BF16, tag="br")
            nc.scalar.activation(ar[:], arp[:], ACTF.Copy)
            nc.scalar.activation(br[:], brp[:], ACTF.Copy)

            r1 = big.tile([P, C, K, K], BF16, tag="r1")
            for mc in range(C):
                cp = psb.tile([P, K, K], F32, tag="pbig")
                for kc in range(C):
                    for nh in range(2):
                        nc.tensor.matmul(
                            cp[:, nh * 16:(nh + 1) * 16, :],
                            w['rt_wd'][:, kc, mc * P:(mc + 1) * P],
                            mulij[:, kc, nh * 16:(nh + 1) * 16, :],
                            start=(kc == 0), stop=(kc == C - 1))
                pre = big.tile([P, K, K], BF16, tag="rpre")
                nc.gpsimd.tensor_tensor(
                    pre[:], ar[:, mc, None, :].broadcast_to([P, K, K]),
                    br[:, mc, :, None].broadcast_to([P, K, K]), op=ALU.add)
                nc.vector.tensor_add(pre[:], pre[:], cp[:])
                nc.scalar.activation(r1[:, mc, :, :], pre[:], ACTF.Relu)

            # feature-major logits [8, 1024]; softmax over the 8 partitions.
            # |logits| << 1 (0.02-scale weights), so skip the max-subtract;
            # exp(logits + b2) fuses into one ACT op via per-partition bias.
            lg = psb.tile([E, 2, 512], F32, tag="pbig")
            for nh in range(2):
                for kc in range(C):
                    nc.tensor.matmul(
                        lg[:, nh, :], w['rt_w2'][:, kc, :],
                        r1[:, kc, :, :].rearrange("p a b -> p (a b)")
                        [:, nh * 512:(nh + 1) * 512],
                        start=(kc == 0), stop=(kc == C - 1))
            ez = wpool.tile([E, E * P], BF16, tag="ez")
            nc.scalar.activation(ez[:].rearrange("e (a b) -> e a b", a=2),
                                 lg[:], ACTF.Exp, bias=w['rt_b2c'][:])
            rc = wpool.tile([1, E * P], F32, tag="rc")
            for nh in range(2):
                smp = ps.tile([1, 512], F32, tag="gen")
                nc.tensor.matmul(smp[:], ones8_bf[:],
                                 ez[:].rearrange("e (a b) -> e a b", a=2)[:, nh, :],
                                 start=True, stop=True)
                nc.scalar.activation(rc[:, nh * 512:(nh + 1) * 512], smp[:],
                                     ACTF.Copy)
            nc.vector.reciprocal(rc[:], rc[:])
            rcb = wpool.tile([1, E * P], BF16, tag="rcb")
            nc.scalar.activation(rcb[:], rc[:], ACTF.Copy)
            rc_bc = wpool.tile([E, E * P], BF16, tag="rc_bc")
            nc.gpsimd.partition_broadcast(rc_bc[:], rcb[:])
            w_all = act.tile([E, E * P], BF16, tag="w_all")
            nc.vector.tensor_mul(w_all[:], ez[:], rc_bc[:])
            nc.sync.dma_start(out=w_dram[t], in_=w_all[:])
            s_sb = sm.tile([E, K], F32, tag="s_sb")
            nc.vector.tensor_reduce(
                s_sb[:], w_all[:].rearrange("e (a b) -> e a b", a=K),
                axis=AX.X, op=ALU.add)
            if dbg_out is not None and t == 0:
                dw = dbgp.tile([E, E * P], F32, tag="dbg_dw")
                nc.scalar.activation(dw[:], w_all[:], ACTF.Copy)
                nc.sync.dma_start(out=dbg_out['dbg_w'][:], in_=dw[:])
                dr1 = dbgp.tile([P, C, K, K], F32, tag="dbg_r1t")
                nc.scalar.activation(dr1[:], r1[:], ACTF.Copy)
                nc.sync.dma_start(out=dbg_out['dbg_r1'][:], in_=dr1[:])

            # all-expert broadcast of w rows across 128 partitions (one DMA)
            wbca = wpool.tile([P, E, E * P], BF16, tag="wbca")
            wsrc = w_dram[t].rearrange("e q -> (e q)")
            wsrc = bass.AP(tensor=wsrc.tensor, offset=wsrc.offset,
                           ap=[[0, P]] + list(wsrc.ap))
            nc.sync.dma_start(out=wbca[:], in_=wsrc)

            # ---------- delta_self ----------
            xfs = sm.tile([P, C, K], F32, tag="xfs")
            layer_norm(h_f, w['fs_g'], w['fs_bb'], C, xfs)
            xu = act.tile([P, 2 * C, K], F32, tag="xu")
            mlp(xfs, w['fs_w1'], w['fs_b1r'], w['fs_w2'], w['fs_b2r'],
                xu[:, 0:C, :])

            # ---------- experts ----------
            dpsa = psd.tile([P, C, K], F32, tag="delta")
            dps = [dpsa[:, 0, :], dpsa[:, 1, :]]
            nc.vector.memset(dpsa[:], 0.0)
            prev_g2 = [None]
            for e in range(E):
                # token-major A^T/B^T: lhsT = h chunk, rhs = expert weights
                abp = ps.tile([K, 2 * D], F32, tag="gen")
                for kc in range(C):
                    nc.tensor.matmul(abp[:], h_b[:, kc, :],
                                     w['ex_wsij'][:, kc, e, :],
                                     start=(kc == 0), stop=False)
                # b1 folded into the B rows: each pair column picks one B row
                nc.tensor.matmul(abp[:], ones_tok_bf[:], w['ex_b1z'][0:1, e, :],
                                 start=False, stop=True)
                ab = sm.tile([2 * K, C, P], BF16, tag="ab")
                nc.scalar.activation(ab[0:K], abp[:, 0:D].rearrange(
                    "k (c p) -> k c p", c=C), ACTF.Copy)
                nc.scalar.activation(ab[K:2 * K], abp[:, D:2 * D].rearrange(
                    "k (c p) -> k c p", c=C), ACTF.Copy)

                prod = big.tile([P, C, K, K], BF16, tag="exprod")
                wv_ = wbca[:, e, :].rearrange("p (b c) -> p b c", b=K)
                for mc in range(C):
                    # pre[d,(j,i)] = A[d,i] + B[d,j] + b1[d] via selector matmul
                    pp_ = psb.tile([P, K, K], F32, tag="pbig")
                    for nh in range(2):
                        nc.tensor.matmul(pp_[:, nh * 16:(nh + 1) * 16, :],
                                         ab[:, mc, :],
                                         sel2[:, nh * 512:(nh + 1) * 512],
                                         start=True, stop=True)
                    if e % 3 == 0:
                        # fused relu+w-mul on DVE: prod = max(pre,0)*w
                        nc.vector.scalar_tensor_tensor(
                            out=prod[:, mc], in0=pp_[:], scalar=0.0, in1=wv_,
                            op0=ALU.max, op1=ALU.mult)
                    else:
                        h1m = big.tile([P, K, K], BF16, tag="exh1")
                        nc.scalar.activation(h1m[:], pp_[:], ACTF.Relu)
                        nc.vector.tensor_mul(prod[:, mc], h1m[:], wv_)
                pr16 = big.tile([P, C, K, 16], BF16, tag="expr16")
                nc.vector.tensor_add(pr16[:], prod[:, :, :, 0:16],
                                     prod[:, :, :, 16:32])
                pr8 = big.tile([P, C, K, 8], BF16, tag="expr8")
                nc.vector.tensor_add(pr8[:], pr16[:, :, :, 0:8],
                                     pr16[:, :, :, 8:16])
                pr4_ = big.tile([P, C, K, 4], BF16, tag="expr4")
                nc.vector.tensor_add(pr4_[:], pr8[:, :, :, 0:4],
                                     pr8[:, :, :, 4:8])
                pr4 = big.tile([P, C, K, NFOLD], BF16, tag="expr2f")
                nc.vector.tensor_add(pr4[:], pr4_[:, :, :, 0:NFOLD],
                                     pr4_[:, :, :, NFOLD:2 * NFOLD])

                for kc in range(C):
                    for mc in range(C):
                        for i in range(NFOLD):
                            mm = nc.tensor.matmul(
                                dps[mc],
                                w['ex_w2'][:, kc, e, mc * P:(mc + 1) * P],
                                pr4[:, kc, :, i],
                                start=False, stop=False,
                                skip_group_check=True)
                            if prev_g2[0] is not None:
                                add_dep_helper(mm.ins, prev_g2[0], sync=False,
                                               reason="g2-weight-run order")
                            prev_g2[0] = mm.ins
                if dbg_out is not None and t == 0 and e == 0:
                    dpr = dbgp.tile([P, C, K, K], F32, tag="dbg_prod_t")
                    nc.scalar.activation(dpr[:], prod[:], ACTF.Copy)
                    nc.sync.dma_start(out=dbg_out['dbg_prod'][:], in_=dpr[:])
                    dd0 = dbgp.tile([P, C, K], F32, tag="dbg_d0_t")
                    nc.scalar.activation(dd0[:, 0, :], dps[0], ACTF.Copy)
                    nc.scalar.activation(dd0[:, 1, :], dps[1], ACTF.Copy)
                    nc.sync.dma_start(out=dbg_out['dbg_d0'][:], in_=dd0[:])
            for mc in range(C):
                nc.tensor.matmul(dps[mc], w['ex_b2t'][:, mc * P:(mc + 1) * P],
                                 s_sb[:], start=False, stop=True,
                                 skip_group_check=True)

            # ---------- update ----------
            nc.scalar.activation(xu[:, C, :], dps[0], ACTF.Copy)
            nc.scalar.activation(xu[:, C + 1, :], dps[1], ACTF.Copy)
            if dbg_out is not None and t == 0:
                nc.sync.dma_start(out=dbg_out['dbg_xu'][:], in_=xu[:])
            xln = sm.tile([P, 2 * C, K], F32, tag="xln")
            layer_norm(xu, w['up_g'], w['up_bb'], 2 * C, xln)
            hnew = act.tile([P, C, K], F32, tag="hnew")
            mlp(xln, w['up_w1'], w['up_b1r'], w['up_w2'], w['up_b2r'], hnew,
                cin=2 * C)

            # ---------- decode ----------
            hd = ps.tile([P, C, K], F32, tag="gen")
            for mc in range(C):
                for kc in range(C):
                    nc.tensor.matmul(hd[:, mc, :],
                                     w['dec_w1'][:, kc, mc * P:(mc + 1) * P],
                                     hnew[:, kc, :], start=(kc == 0), stop=False)
                bias_fold(hd[:, mc, :], w['dec_b1r'], mc, stop=True)
            hds = sm.tile([P, C, K], F32, tag="dec_h")
            nc.scalar.activation(hds[:], hd[:], ACTF.Relu)
            pp = ps.tile([DS, K], F32, tag="gen")
            for kc in range(C):
                nc.tensor.matmul(pp[:], w['dec_w2'][:, kc, :], hds[:, kc, :],
                                 start=(kc == 0), stop=False)
            nc.tensor.matmul(pp[:], w['dec_b2r'][:], ones_tok[:],
                             start=False, stop=True)
            pred_sb = act.tile([DS, K], F32, tag="pred_sb")
            nc.scalar.activation(pred_sb[:], pp[:], ACTF.Copy)
            nc.sync.dma_start(out=pred_out[t].rearrange("k s -> s k"),
                              in_=pred_sb[:])

            if t < n_steps - 1:
                encode(pred_sb)


# --------------------------------------------------------------------------
# host wrapper
# --------------------------------------------------------------------------

_PROG_CACHE = {}


def _get_program(n_steps):
    if n_steps not in _PROG_CACHE:
        _PROG_CACHE[n_steps] = build_program(n_steps)
    return _PROG_CACHE[n_steps]


def kernel(gt_states, params, rollout_steps):
    from concourse.bass_utils import run_bass_kernel_spmd

    gt = np.asarray(gt_states, np.float32)
    B, T, K_, Ds_ = gt.shape
    n_steps = min(T - 1, int(rollout_steps))
    nc = _get_program(n_steps)

    shared = prep_params(params)
    in_maps = []
    for b in range(B):
        m = dict(shared)
        m['x0t'] = np.ascontiguousarray(gt[b, 0].T)   # [8, 32]
        in_maps.append(m)

    res = run_bass_kernel_spmd(nc, in_maps, core_ids=list(range(B)))
    pred = np.stack([res.results[b]['pred'] for b in range(B)], 0)
    target = gt[:, 1:n_steps + 1]
    return pred.astype(np.float32), target
